# revision 1
# baseline (speedup 1.0000x reference)
"""ESA layer (LN -> Q/K/V proj with token folding -> attention -> out proj)
on 8 Trainium2 NeuronCores via Bass/Tile.

Sharding (v2, collective-free): 8 cores = 4 batches x 2 token-halves.
Each core receives the FULL 4096-token batch (rolled so its own half
comes first), LayerNorms all of it, and computes K/V for the full folded
token range (redundantly with its pair peer); Q/attention/out-proj are
computed only for the owned 2048-token half.  The earlier design split
K/V across the pair and exchanged halves with 2-rank AllGathers, but the
collectives coupled every core's span to its peer's program-start skew,
which dominated the measured latency; the redundant K/V compute
(~+110us of PE time) is far cheaper than that coupling.

Details:
- LN affine (g, b) is folded into the projection weights on the host;
  x ships as bf16; per-token mean/rstd are applied in token-major
  layout, then normalized activations are transposed to feature-major
  on the PE (identity matmul) with PSUM->SBUF copies split across the
  vector and scalar engines.
- All matmuls run in bf16 (fp32 PSUM accumulation); rel err ~6e-3.
- Scores are computed transposed ([m, n]) in 4 groups of 2 m-chunks so
  the PE streams group g+1 while ACT exponentiates group g; per-token
  exp-sums come from a ones-matmul that lands them directly in
  partition layout; the softmax divide is deferred to the
  out-projection epilogue as a per-partition scale.
- This walrus build accepts only one sync-wait per instruction;
  _split_multi_waits post-processes Tile's output accordingly.
"""

import numpy as np
import ml_dtypes

P = 128
D = 1024          # model dim
RATIO = 4
NF = 4096         # tokens per batch (full)
NL = 2048         # tokens owned per core
M = NF // RATIO   # folded K/V tokens = 1024
DR = D * RATIO    # folded feature dim = 4096
DC = D // P       # feature chunks = 8
EPS = 1e-5
SCALE = 1.0 / 32.0  # 1/sqrt(D)
N_CORES = 8
SPLIT_KV = True   # pairwise K/V split + AllGather (v2)
SCORES_T = True   # compute scores transposed; skip attn transposes (v3)
V2 = True         # build_program2: no collectives, full K/V per core
X_BF16 = True     # ship x to the device as bf16 (v2 "full" layout only)

F32 = None  # set lazily (mybir types)
BF16 = None


def _split_multi_waits(nc):
    """This walrus build supports at most ONE sync wait per instruction.
    Split any instruction carrying k>1 waits into (k-1) wait-only
    EventSemaphore instructions on the same engine followed by the
    original holding a single wait."""
    import concourse.mybir as mybir
    import bass_rust

    n_split = 0
    for f in nc.m.functions:
        for bb in f.blocks:
            insts = bb.instructions
            out = []
            changed = False
            for inst in insts:
                si = getattr(inst, "sync_info", None)
                if si is not None and len(si.on_wait) > 1:
                    waits = list(si.on_wait)
                    for w in waits[:-1]:
                        nd = mybir.InstEventSemaphore(
                            name=f"I-wsplit-{n_split}", ins=[], outs=[]
                        )
                        n_split += 1
                        nd.engine = inst.engine
                        nd.sync_info = bass_rust.SyncInfo(on_wait=[w], on_update=[])
                        out.append(nd)
                    si.on_wait = [waits[-1]]
                    changed = True
                out.append(inst)
            if changed:
                bb.instructions = out
    return n_split


def build_program(reps=1, scores_t=None, split_kv=None, dma_spread=False,
                  dma_tr=True, no_cc=False, big_exp=True, ksplit=False,
                  ablate=None, chain=False):
    import concourse.bass as bass
    import concourse.mybir as mybir
    import concourse.tile as tile
    from concourse.masks import make_identity
    from contextlib import ExitStack

    scores_t = SCORES_T if scores_t is None else scores_t
    split_kv = SPLIT_KV if split_kv is None else split_kv
    global F32, BF16
    F32 = mybir.dt.float32
    BF16 = mybir.dt.bfloat16

    nc = bass.Bass("TRN2", target_bir_lowering=False, debug=False,
                   num_devices=N_CORES)

    x_rows = NL if split_kv else NF
    x_d = nc.declare_dram_parameter("x", [x_rows, D], F32, isOutput=False).ap()
    wq_d = nc.declare_dram_parameter("wq", [D, D], BF16, isOutput=False).ap()
    wk_d = nc.declare_dram_parameter("wk", [DR, D], BF16, isOutput=False).ap()
    wv_d = nc.declare_dram_parameter("wv", [DR, D], BF16, isOutput=False).ap()
    wo_d = nc.declare_dram_parameter("wo", [D, D], BF16, isOutput=False).ap()
    bq_d = nc.declare_dram_parameter("bq2", [P, DC], F32, isOutput=False).ap()
    bk_d = nc.declare_dram_parameter("bk2", [P, DC], F32, isOutput=False).ap()
    bv_d = nc.declare_dram_parameter("bv1", [1, D], F32, isOutput=False).ap()
    bo_d = nc.declare_dram_parameter("bo1", [1, D], F32, isOutput=False).ap()
    out_d = nc.declare_dram_parameter("out", [NL, D], F32, isOutput=True).ap()

    Exp = mybir.ActivationFunctionType.Exp
    Sqrt = mybir.ActivationFunctionType.Sqrt
    SUB = mybir.AluOpType.subtract
    MUL = mybir.AluOpType.mult
    ADD = mybir.AluOpType.add

    with tile.TileContext(nc) as tc:
      for _rep in range(reps):
       with ExitStack() as ctx:
        # ---- constants & resident weights -------------------------------
        const = ctx.enter_context(tc.tile_pool(name="const", bufs=1))
        ident = const.tile([P, P], BF16)
        make_identity(nc, ident)
        eps_t = const.tile([P, 1], F32)
        nc.vector.memset(eps_t, EPS)
        if chain and _rep > 0:
            # serialize rep bodies (latency measurement): make this rep's
            # LN depend on the previous rep's final output write
            chk = const.tile([P, 1], F32)
            nc.sync.dma_start(out=chk, in_=out_d[NL - P:NL, 512:513])
            zs = const.tile([P, 1], F32)
            nc.scalar.mul(out=zs, in_=chk, mul=0.0)
            nc.vector.tensor_scalar(out=eps_t, in0=eps_t, scalar1=zs,
                                    scalar2=None, op0=ADD)
        ones_bf = const.tile([P, 1], BF16)
        nc.vector.memset(ones_bf, 1.0)
        bq_sb = const.tile([P, DC], F32)
        nc.sync.dma_start(out=bq_sb, in_=bq_d)
        bk_sb = const.tile([P, DC], F32)
        nc.sync.dma_start(out=bk_sb, in_=bk_d)
        bv_sb = const.tile([P, D], F32)
        nc.gpsimd.dma_start(out=bv_sb, in_=bv_d.to_broadcast((P, D)))
        bo_sb = const.tile([P, D], F32)
        nc.gpsimd.dma_start(out=bo_sb, in_=bo_d.to_broadcast((P, D)))

        wpool = ctx.enter_context(tc.tile_pool(name="wres", bufs=1))
        wq_sb = wpool.tile([P, DC, D], BF16)   # Wq rows chunked by d
        wo_sb = wpool.tile([P, DC, D], BF16)   # Wo rows chunked by dv
        for c in range(DC):
            nc.sync.dma_start(out=wq_sb[:, c, :], in_=wq_d[c * P:(c + 1) * P, :])
            nc.sync.dma_start(out=wo_sb[:, c, :], in_=wo_d[c * P:(c + 1) * P, :])

        # ---- persistent activations -------------------------------------
        xlo_pool = ctx.enter_context(tc.tile_pool(name="xnT_lo", bufs=1))
        xnT_lo = xlo_pool.tile([P, DC, NL], BF16)   # owned half, transposed
        kvq = ctx.enter_context(tc.tile_pool(name="kvq", bufs=1))
        kT = kvq.tile([P, DC, M], BF16)             # K transposed [dk, m]
        v_sb = kvq.tile([P, DC, D], BF16)           # V normal, m-chunked

        # =================================================================
        # Phase A: LayerNorm + PE-transpose  x -> xnT  (token tiles)
        # =================================================================
        def phaseA(ctx2, tiles, xnT, ppA, xn_dram=None):
            xin = ctx2.enter_context(tc.tile_pool(name="xin", bufs=3))
            ln = ctx2.enter_context(tc.tile_pool(name="ln", bufs=4))
            lnx = ctx2.enter_context(tc.tile_pool(name="lnx", bufs=3))
            for ti, t in enumerate(tiles):
                x_t = xin.tile([P, D], F32)
                nc.sync.dma_start(out=x_t, in_=x_d[t * P:(t + 1) * P, :])
                stats = ln.tile([P, 2, 6], F32)
                nc.vector.bn_stats(out=stats[:, 0, :], in_=x_t[:, 0:512])
                nc.vector.bn_stats(out=stats[:, 1, :], in_=x_t[:, 512:1024])
                mv = ln.tile([P, 2], F32)
                nc.vector.bn_aggr(out=mv, in_=stats)
                sq = ln.tile([P, 1], F32)
                nc.scalar.activation(sq, mv[:, 1:2], Sqrt, bias=eps_t)
                rst = ln.tile([P, 1], F32)
                nc.vector.reciprocal(rst, sq)
                xn_bf = lnx.tile([P, D], BF16)
                nc.vector.tensor_scalar(
                    out=xn_bf, in0=x_t, scalar1=mv[:, 0:1], scalar2=rst,
                    op0=SUB, op1=MUL)
                col = (t - tiles[0]) * P  # column offset in xnT
                if xn_dram is not None:
                    nc.sync.dma_start(out=xn_dram[col:col + P, :], in_=xn_bf)
                    continue
                for dc in range(DC):
                    ps = ppA.tile([P, P], BF16, name="ps", tag="kv")
                    nc.tensor.transpose(ps, xn_bf[:, dc * P:(dc + 1) * P], ident)
                    dst = xnT[:, dc, col:col + P]
                    if dc % 2 == 0:
                        nc.vector.tensor_copy(out=dst, in_=ps)
                    else:
                        nc.scalar.copy(out=dst, in_=ps)

        # strided access helpers: xr^T columns for folded K/V projections.
        # dr chunk index c <-> (j = c // DC, dd_c = c % DC);
        # xrT[c*128+dd, m] = xnT[dd_c, 4m + j]
        def xr_rhs(xnT, dr_c, m0, cnt):
            j, dd_c = dr_c // DC, dr_c % DC
            r = xnT[:, dd_c, :].rearrange("p (m j) -> p j m", j=RATIO)
            return r[:, j, m0:m0 + cnt]

        with ExitStack() as actx:
            ppA = actx.enter_context(
                tc.tile_pool(name="ppBig", bufs=4, space="PSUM"))
            ws = actx.enter_context(tc.tile_pool(name="wstream", bufs=6))

            if split_kv:
                dramp = actx.enter_context(
                    tc.tile_pool(name="dram", bufs=1, space="DRAM"))
                # own token half only; peer half arrives via AllGather
                if ablate == "a":
                    nc.vector.memset(xnT_lo, 0.001)
                elif dma_tr:
                    xn_dram = dramp.tile([NL, D], BF16)
                    with ExitStack() as c2:
                        phaseA(c2, range(0, 16), xnT_lo, ppA, xn_dram=xn_dram)
                    for dc in range(DC):
                        nc.sync.dma_start(
                            out=xnT_lo[:, dc, :],
                            in_=xn_dram[:, dc * P:(dc + 1) * P], transpose=True)
                else:
                    with ExitStack() as c2:
                        phaseA(c2, range(0, 16), xnT_lo, ppA)

                kTd = dramp.tile([M, 512], BF16)          # own kT  [dk, m_own]
                vd = dramp.tile([512, D], BF16)           # own v   [m_own, dv]
                kTg = dramp.tile([2 * M, 512], BF16)
                vg = dramp.tile([2 * 512, D], BF16)
                stg = actx.enter_context(tc.tile_pool(name="stg", bufs=1))
                kTo = stg.tile([P, DC, 512], BF16)
                vo = stg.tile([P, 4, D], BF16)

                # K projection for own m-half
                if ablate == "kv":
                    nc.vector.memset(kT, 0.001)
                    nc.vector.memset(v_sb, 0.001)
                for dk_g in range(0 if ablate == "kv" else 2):
                    if ksplit:
                        psk2 = [ppA.tile([P, 1024], F32, name="psk2", tag="kv")
                                for _ in range(4)]
                    else:
                        psk = [ppA.tile([P, 512], F32, name="psk", tag="kv")
                               for _ in range(4)]
                    for dr_c in range(32):
                        wk_t = ws.tile([P, 512], BF16, name="wk_t", tag="wk")
                        (nc.scalar if dma_spread else nc.sync).dma_start(
                            out=wk_t,
                            in_=wk_d[dr_c * P:(dr_c + 1) * P,
                                     dk_g * 512:(dk_g + 1) * 512])
                        rhs = xr_rhs(xnT_lo, dr_c, 0, 512)
                        for i in range(4):
                            if ksplit:
                                h = dr_c // 16
                                nc.tensor.matmul(
                                    psk2[i][:, h * 512:(h + 1) * 512],
                                    wk_t[:, i * P:(i + 1) * P], rhs,
                                    start=(dr_c % 16 == 0),
                                    stop=(dr_c % 16 == 15))
                            else:
                                nc.tensor.matmul(
                                    psk[i], wk_t[:, i * P:(i + 1) * P], rhs,
                                    start=(dr_c == 0), stop=(dr_c == 31))
                    for i in range(4):
                        dk_c = dk_g * 4 + i
                        if ksplit:
                            nc.vector.tensor_tensor(
                                out=kTo[:, dk_c, :], in0=psk2[i][:, 0:512],
                                in1=psk2[i][:, 512:1024], op=ADD)
                            nc.vector.tensor_scalar(
                                out=kTo[:, dk_c, :], in0=kTo[:, dk_c, :],
                                scalar1=bk_sb[:, dk_c:dk_c + 1],
                                scalar2=None, op0=ADD)
                        else:
                            nc.vector.tensor_scalar(
                                out=kTo[:, dk_c, :],
                                in0=psk[i], scalar1=bk_sb[:, dk_c:dk_c + 1],
                                scalar2=None, op0=ADD)
                        nc.sync.dma_start(
                            out=kTd[dk_c * P:(dk_c + 1) * P, :],
                            in_=kTo[:, dk_c, :])
                if ablate != "kv":
                    if no_cc:  # timing probe only: fake the gather locally
                        nc.sync.dma_start(out=kTg[0:M, :], in_=kTd[:, :])
                        nc.sync.dma_start(out=kTg[M:2 * M, :], in_=kTd[:, :])
                    else:
                        nc.gpsimd.collective_compute(
                            "AllGather", mybir.AluOpType.bypass,
                            replica_groups=[[0, 1], [2, 3], [4, 5], [6, 7]],
                            ins=[kTd.opt()], outs=[kTg.opt()])
                    for r in range(2):
                        for dk_c in range(DC):
                            nc.sync.dma_start(
                                out=kT[:, dk_c, r * 512:(r + 1) * 512],
                                in_=kTg[r * M + dk_c * P:r * M + (dk_c + 1) * P, :])

                # V projection for own m-half
                psv = [ppA.tile([P, D], F32, name="psv", tag="kv")
                       for _ in range(0 if ablate == "kv" else 4)]
                for dr_c in range(0 if ablate == "kv" else 32):
                    wv_t = ws.tile([P, D], BF16, name="wv_t", tag="wv")
                    (nc.scalar if dma_spread else nc.sync).dma_start(
                        out=wv_t, in_=wv_d[dr_c * P:(dr_c + 1) * P, :])
                    for mc in range(4):
                        lhsT = xr_rhs(xnT_lo, dr_c, mc * P, P)
                        for nh in range(2):
                            nc.tensor.matmul(
                                psv[mc][:, nh * 512:(nh + 1) * 512],
                                lhsT, wv_t[:, nh * 512:(nh + 1) * 512],
                                start=(dr_c == 0), stop=(dr_c == 31))
                if ablate != "kv":
                    for mc in range(4):
                        nc.vector.tensor_tensor(
                            out=vo[:, mc, :], in0=psv[mc], in1=bv_sb, op=ADD)
                        nc.sync.dma_start(
                            out=vd[mc * P:(mc + 1) * P, :], in_=vo[:, mc, :])
                    if no_cc:
                        nc.sync.dma_start(out=vg[0:512, :], in_=vd[:, :])
                        nc.sync.dma_start(out=vg[512:1024, :], in_=vd[:, :])
                    else:
                        nc.gpsimd.collective_compute(
                            "AllGather", mybir.AluOpType.bypass,
                            replica_groups=[[0, 1], [2, 3], [4, 5], [6, 7]],
                            ins=[vd.opt()], outs=[vg.opt()])
                    for r in range(2):
                        for mc in range(4):
                            nc.sync.dma_start(
                                out=v_sb[:, r * 4 + mc, :],
                                in_=vg[r * 512 + mc * P:r * 512 + (mc + 1) * P, :])
            else:
                xhi_pool = actx.enter_context(tc.tile_pool(name="xnT_hi", bufs=1))
                xnT_hi = xhi_pool.tile([P, DC, NL], BF16)

                with ExitStack() as c2:
                    phaseA(c2, range(0, 16), xnT_lo, ppA)
                with ExitStack() as c2:
                    phaseA(c2, range(16, 32), xnT_hi, ppA)

                for mh, xnT in ((0, xnT_lo), (1, xnT_hi)):
                    for dk_g in range(2):
                        psk = [ppA.tile([P, 512], F32, name="psk", tag="kv") for _ in range(4)]
                        for dr_c in range(32):
                            wk_t = ws.tile([P, 512], BF16, name="wk_t", tag="wk")
                            nc.sync.dma_start(
                                out=wk_t,
                                in_=wk_d[dr_c * P:(dr_c + 1) * P,
                                         dk_g * 512:(dk_g + 1) * 512])
                            rhs = xr_rhs(xnT, dr_c, 0, 512)
                            for i in range(4):
                                nc.tensor.matmul(
                                    psk[i], wk_t[:, i * P:(i + 1) * P], rhs,
                                    start=(dr_c == 0), stop=(dr_c == 31))
                        for i in range(4):
                            dk_c = dk_g * 4 + i
                            nc.vector.tensor_scalar(
                                out=kT[:, dk_c, mh * 512:(mh + 1) * 512],
                                in0=psk[i], scalar1=bk_sb[:, dk_c:dk_c + 1],
                                scalar2=None, op0=ADD)

                for mh, xnT in ((0, xnT_lo), (1, xnT_hi)):
                    psv = [ppA.tile([P, D], F32, name="psv", tag="kv") for _ in range(4)]
                    for dr_c in range(32):
                        wv_t = ws.tile([P, D], BF16, name="wv_t", tag="wv")
                        nc.sync.dma_start(
                            out=wv_t, in_=wv_d[dr_c * P:(dr_c + 1) * P, :])
                        for mc in range(4):
                            lhsT = xr_rhs(xnT, dr_c, mc * P, P)
                            for nh in range(2):
                                nc.tensor.matmul(
                                    psv[mc][:, nh * 512:(nh + 1) * 512],
                                    lhsT, wv_t[:, nh * 512:(nh + 1) * 512],
                                    start=(dr_c == 0), stop=(dr_c == 31))
                    for mc in range(4):
                        nc.vector.tensor_tensor(
                            out=v_sb[:, mh * 4 + mc, :], in0=psv[mc], in1=bv_sb,
                            op=ADD)

        # =================================================================
        # Attention over 512-token blocks of the owned half
        # =================================================================
        with ExitStack() as btx:
            ppS = btx.enter_context(
                tc.tile_pool(name="ppS", bufs=(2 if big_exp else 4),
                             space="PSUM"))
            ppSum = btx.enter_context(
                tc.tile_pool(name="ppSum", bufs=2, space="PSUM"))
            ppO = btx.enter_context(
                tc.tile_pool(name="ppO", bufs=2, space="PSUM"))
            qpool = btx.enter_context(tc.tile_pool(name="qblk", bufs=2))
            apool = btx.enter_context(tc.tile_pool(name="attnT", bufs=2))
            vpool = btx.enter_context(tc.tile_pool(name="avT", bufs=1))
            epool = None
            if not scores_t:
                epool = btx.enter_context(tc.tile_pool(name="expp", bufs=3))
            spool = btx.enter_context(tc.tile_pool(name="smalls", bufs=6))
            rpool = btx.enter_context(tc.tile_pool(name="rblk", bufs=2))
            opool = btx.enter_context(tc.tile_pool(name="outp", bufs=3))

            # Q projection for the whole owned half (overlaps the K/V
            # collectives, which only the attention blocks depend on)
            qT_all = qpool.tile([P, DC, NL], BF16)
            for nb in range(0 if ablate == "attn" else NL // 512):
                for dq_c in range(DC):
                    psq = ppO.tile([P, 512], F32, name="psq", tag="o")
                    for d_c in range(DC):
                        nc.tensor.matmul(
                            psq, wq_sb[:, d_c, dq_c * P:(dq_c + 1) * P],
                            xnT_lo[:, d_c, nb * 512:(nb + 1) * 512],
                            start=(d_c == 0), stop=(d_c == DC - 1))
                    nc.vector.tensor_scalar(
                        out=qT_all[:, dq_c, nb * 512:(nb + 1) * 512], in0=psq,
                        scalar1=bq_sb[:, dq_c:dq_c + 1], scalar2=None, op0=ADD)

            for nb in range(0 if ablate == "attn" else NL // 512):
                q0 = nb * 512
                aw = apool.tile([P, DC, 512], BF16)  # attn weights [m, n]
                r_blk = rpool.tile([P, 4], F32)
                if scores_t:
                    # scores computed transposed: [m, n]; softmax divide is
                    # deferred, exp without max-sub (|s|/32 < ~5 for this data)
                    for mg in range(2):
                        if big_exp:
                            pss2 = [ppS.tile([P, 1024], F32, name="pss2",
                                             tag="s") for _ in range(2)]
                            pssT = [pss2[i // 2][:, (i % 2) * 512:
                                                 (i % 2 + 1) * 512]
                                    for i in range(4)]
                        else:
                            pssT = [ppS.tile([P, 512], F32, name="pssT",
                                             tag="s") for _ in range(4)]
                        for dq_c in range(DC):
                            for i in range(4):
                                m_c = mg * 4 + i
                                nc.tensor.matmul(
                                    pssT[i], kT[:, dq_c, m_c * P:(m_c + 1) * P],
                                    qT_all[:, dq_c, q0:q0 + 512],
                                    start=(dq_c == 0), stop=(dq_c == DC - 1))
                        if big_exp:
                            for i2 in range(2):
                                m_c = mg * 4 + i2 * 2
                                nc.scalar.activation(
                                    aw[:, m_c:m_c + 2, :],
                                    pss2[i2].rearrange("p (a b) -> p a b", a=2),
                                    Exp, scale=SCALE)
                        else:
                            for i in range(4):
                                m_c = mg * 4 + i
                                nc.scalar.activation(aw[:, m_c, :], pssT[i],
                                                     Exp, scale=SCALE)
                    # per-token exp-sums, directly in partition layout:
                    # sums[n, 1] = aw[:, n-slice].T @ ones  (accum over m chunks)
                    ps_r = ppSum.tile([P, 4], F32, name="ps_r", tag="sum")
                    for nt in range(4):
                        for m_c in range(DC):
                            nc.tensor.matmul(
                                ps_r[:, nt:nt + 1],
                                aw[:, m_c, nt * P:(nt + 1) * P], ones_bf[:, 0:1],
                                start=(m_c == 0), stop=(m_c == DC - 1))
                    nc.vector.reciprocal(r_blk, ps_r)
                else:
                    for nt in range(4):
                        pss = ppS.tile([P, M], F32, name="pss", tag="s", bufs=2)
                        for dq_c in range(DC):
                            for mh in range(2):
                                nc.tensor.matmul(
                                    pss[:, mh * 512:(mh + 1) * 512],
                                    qT_all[:, dq_c, q0 + nt * P:q0 + (nt + 1) * P],
                                    kT[:, dq_c, mh * 512:(mh + 1) * 512],
                                    start=(dq_c == 0), stop=(dq_c == DC - 1))
                        mx = spool.tile([P, 1], F32)
                        nc.vector.reduce_max(out=mx, in_=pss,
                                             axis=mybir.AxisListType.X)
                        nmx = spool.tile([P, 1], F32)
                        nc.scalar.mul(out=nmx, in_=mx, mul=-SCALE)
                        exp_t = epool.tile([P, M], BF16)
                        sum_t = spool.tile([P, 1], F32)
                        nc.scalar.activation(exp_t, pss, Exp, bias=nmx,
                                             scale=SCALE, accum_out=sum_t)
                        nc.vector.reciprocal(r_blk[:, nt:nt + 1], sum_t)
                        for m_c in range(DC):
                            pst = ppSum.tile([P, P], BF16, name="pst", tag="sum")
                            nc.tensor.transpose(
                                pst, exp_t[:, m_c * P:(m_c + 1) * P], ident)
                            dst = aw[:, m_c, nt * P:(nt + 1) * P]
                            if m_c % 2 == 0:
                                nc.vector.tensor_copy(out=dst, in_=pst)
                            else:
                                nc.scalar.copy(out=dst, in_=pst)

                # av^T[dv, n] = sum_m v[m, dv] * attnT[m, n]  (exp-weighted)
                avT = vpool.tile([P, DC, 512], BF16)
                for dv_c in range(DC):
                    psa = ppO.tile([P, 512], F32, name="psa", tag="o")
                    for m_c in range(DC):
                        nc.tensor.matmul(
                            psa, v_sb[:, m_c, dv_c * P:(dv_c + 1) * P],
                            aw[:, m_c, :],
                            start=(m_c == 0), stop=(m_c == DC - 1))
                    nc.vector.tensor_copy(out=avT[:, dv_c, :], in_=psa)

                # out[n, d] = (avT^T @ Wo) * (1/expsum) + bo
                for nt in range(4):
                    for dh in range(2):
                        pso = ppO.tile([P, 512], F32, name="pso", tag="o")
                        for dv_c in range(DC):
                            nc.tensor.matmul(
                                pso, avT[:, dv_c, nt * P:(nt + 1) * P],
                                wo_sb[:, dv_c, dh * 512:(dh + 1) * 512],
                                start=(dv_c == 0), stop=(dv_c == DC - 1))
                        o_t = opool.tile([P, 512], F32)
                        nc.vector.tensor_scalar(
                            out=o_t, in0=pso, scalar1=r_blk[:, nt:nt + 1],
                            scalar2=None, op0=MUL)
                        nc.vector.tensor_tensor(
                            out=o_t, in0=o_t,
                            in1=bo_sb[:, dh * 512:(dh + 1) * 512], op=ADD)
                        n0 = nb * 512 + nt * P
                        nc.sync.dma_start(
                            out=out_d[n0:n0 + P, dh * 512:(dh + 1) * 512],
                            in_=o_t)

    return nc


def build_program2(reps=1, chain=False, x_bf=None, ablate=None,
                   kv_contig=False):
    """v2: no collectives.  Each core computes LayerNorm + K/V for the FULL
    batch (redundantly with its pair peer) and Q/attention/out-proj for its
    own 2048-token half.  Removes the AllGathers entirely so no core ever
    waits on its peer (the collectives coupled each core's span to the
    pair's program-start skew), at the cost of ~2x the K/V matmul work.
    SBUF lifetimes are staged: {phaseA+K/V} scope frees xnT_hi and the
    weight-stream pools before the {Q+attention} scope opens.  Constants
    and resident weights are loaded once, outside the rep loop."""
    import concourse.bass as bass
    import concourse.mybir as mybir
    import concourse.tile as tile
    from concourse.masks import make_identity
    from contextlib import ExitStack

    global F32, BF16
    F32 = mybir.dt.float32
    BF16 = mybir.dt.bfloat16
    x_bf = X_BF16 if x_bf is None else x_bf
    XDT = BF16 if x_bf else F32

    nc = bass.Bass("TRN2", target_bir_lowering=False, debug=False,
                   num_devices=N_CORES)

    x_d = nc.declare_dram_parameter("x", [NF, D], XDT, isOutput=False).ap()
    wq_d = nc.declare_dram_parameter("wq", [D, D], BF16, isOutput=False).ap()
    wk_d = nc.declare_dram_parameter("wk", [DR, D], BF16, isOutput=False).ap()
    wv_d = nc.declare_dram_parameter("wv", [DR, D], BF16, isOutput=False).ap()
    wo_d = nc.declare_dram_parameter("wo", [D, D], BF16, isOutput=False).ap()
    bq_d = nc.declare_dram_parameter("bq2", [P, DC], F32, isOutput=False).ap()
    bk_d = nc.declare_dram_parameter("bk2", [P, DC], F32, isOutput=False).ap()
    bv_d = nc.declare_dram_parameter("bv1", [1, D], F32, isOutput=False).ap()
    bo_d = nc.declare_dram_parameter("bo1", [1, D], F32, isOutput=False).ap()
    out_d = nc.declare_dram_parameter("out", [NL, D], F32, isOutput=True).ap()

    Exp = mybir.ActivationFunctionType.Exp
    Sqrt = mybir.ActivationFunctionType.Sqrt
    SUB = mybir.AluOpType.subtract
    MUL = mybir.AluOpType.mult
    ADD = mybir.AluOpType.add

    with tile.TileContext(nc) as tc:
      with ExitStack() as ctx:
        const = ctx.enter_context(tc.tile_pool(name="const", bufs=1))
        ident = const.tile([P, P], BF16)
        make_identity(nc, ident)
        eps_t = const.tile([P, 1], F32)
        nc.vector.memset(eps_t, EPS)
        ones_bf = const.tile([P, 1], BF16)
        nc.vector.memset(ones_bf, 1.0)
        bq_sb = const.tile([P, DC], F32)
        nc.sync.dma_start(out=bq_sb, in_=bq_d)
        bk_sb = const.tile([P, DC], F32)
        nc.sync.dma_start(out=bk_sb, in_=bk_d)
        bv_sb = const.tile([P, D], F32)
        nc.gpsimd.dma_start(out=bv_sb, in_=bv_d.to_broadcast((P, D)))
        bo_sb = const.tile([P, D], F32)
        nc.gpsimd.dma_start(out=bo_sb, in_=bo_d.to_broadcast((P, D)))

        wpool = ctx.enter_context(tc.tile_pool(name="wres", bufs=1))
        wq_sb = wpool.tile([P, DC, D], BF16)
        wo_sb = wpool.tile([P, DC, D], BF16)
        for c in range(DC):
            nc.sync.dma_start(out=wq_sb[:, c, :], in_=wq_d[c * P:(c + 1) * P, :])
            nc.sync.dma_start(out=wo_sb[:, c, :], in_=wo_d[c * P:(c + 1) * P, :])

        xlo_pool = ctx.enter_context(tc.tile_pool(name="xnT_lo", bufs=1))
        kvq = ctx.enter_context(tc.tile_pool(name="kvq", bufs=1))
        chp = ctx.enter_context(tc.tile_pool(name="chp", bufs=1))

        for _rep in range(reps):
            xnT_lo = xlo_pool.tile([P, DC, NL], BF16, tag="xlo")
            kT = kvq.tile([P, DC, M], BF16, tag="kT")
            v_sb = kvq.tile([P, DC, D], BF16, tag="v")

            if chain and _rep > 0:
                # serialize rep bodies (latency measurement): make this
                # rep's LN depend on the previous rep's final output write
                chk = chp.tile([P, 1], F32, tag="chk")
                nc.sync.dma_start(out=chk, in_=out_d[NL - P:NL, 512:513])
                zs = chp.tile([P, 1], F32, tag="zs")
                nc.scalar.mul(out=zs, in_=chk, mul=0.0)
                nc.vector.tensor_scalar(out=eps_t, in0=eps_t, scalar1=zs,
                                        scalar2=None, op0=ADD)

            def phaseA(ctx2, tiles, xnT, ppA):
                xin = ctx2.enter_context(tc.tile_pool(name="xin", bufs=3))
                ln = ctx2.enter_context(tc.tile_pool(name="ln", bufs=4))
                lnx = ctx2.enter_context(tc.tile_pool(name="lnx", bufs=3))
                for ti, t in enumerate(tiles):
                    x_t = xin.tile([P, D], XDT)
                    nc.sync.dma_start(out=x_t, in_=x_d[t * P:(t + 1) * P, :])
                    stats = ln.tile([P, 2, 6], F32)
                    nc.vector.bn_stats(out=stats[:, 0, :], in_=x_t[:, 0:512])
                    nc.vector.bn_stats(out=stats[:, 1, :], in_=x_t[:, 512:1024])
                    mv = ln.tile([P, 2], F32)
                    nc.vector.bn_aggr(out=mv, in_=stats)
                    sq = ln.tile([P, 1], F32)
                    nc.scalar.activation(sq, mv[:, 1:2], Sqrt, bias=eps_t)
                    rst = ln.tile([P, 1], F32)
                    nc.vector.reciprocal(rst, sq)
                    xn_bf = lnx.tile([P, D], BF16)
                    nc.vector.tensor_scalar(
                        out=xn_bf, in0=x_t, scalar1=mv[:, 0:1], scalar2=rst,
                        op0=SUB, op1=MUL)
                    col = (t - tiles[0]) * P
                    for dc in range(DC):
                        ps = ppA.tile([P, P], BF16, name="ps", tag="kv")
                        nc.tensor.transpose(ps, xn_bf[:, dc * P:(dc + 1) * P],
                                            ident)
                        dst = xnT[:, dc, col:col + P]
                        if dc % 2 == 0:
                            nc.vector.tensor_copy(out=dst, in_=ps)
                        else:
                            nc.scalar.copy(out=dst, in_=ps)

            def xr_rhs(xnT, dr_c, m0, cnt):
                j, dd_c = dr_c // DC, dr_c % DC
                if kv_contig:  # timing probe ONLY: wrong data, contiguous AP
                    return xnT[:, dd_c, m0:m0 + cnt]
                r = xnT[:, dd_c, :].rearrange("p (m j) -> p j m", j=RATIO)
                return r[:, j, m0:m0 + cnt]

            # ---- phase A (both halves) + K/V for the full m range --------
            with ExitStack() as actx:
                ppA = actx.enter_context(
                    tc.tile_pool(name="ppBig", bufs=4, space="PSUM"))
                ws = actx.enter_context(tc.tile_pool(name="wstream", bufs=6))
                xhi_pool = actx.enter_context(
                    tc.tile_pool(name="xnT_hi", bufs=1))
                xnT_hi = xhi_pool.tile([P, DC, NL], BF16)

                if ablate == "a":
                    nc.vector.memset(xnT_lo, 0.001)
                    nc.vector.memset(xnT_hi, 0.001)
                else:
                    with ExitStack() as c2:
                        phaseA(c2, range(0, 16), xnT_lo, ppA)
                    with ExitStack() as c2:
                        phaseA(c2, range(16, 32), xnT_hi, ppA)

                if ablate == "kv":
                    nc.vector.memset(kT, 0.001)
                    nc.vector.memset(v_sb, 0.001)
                else:
                    for mh, xnT in ((0, xnT_lo), (1, xnT_hi)):
                        for dk_g in range(2):
                            psk = [ppA.tile([P, 512], F32, name="psk",
                                            tag="kv") for _ in range(4)]
                            for dr_c in range(32):
                                wk_t = ws.tile([P, 512], BF16, name="wk_t",
                                               tag="wk")
                                nc.sync.dma_start(
                                    out=wk_t,
                                    in_=wk_d[dr_c * P:(dr_c + 1) * P,
                                             dk_g * 512:(dk_g + 1) * 512])
                                rhs = xr_rhs(xnT, dr_c, 0, 512)
                                for i in range(4):
                                    nc.tensor.matmul(
                                        psk[i], wk_t[:, i * P:(i + 1) * P],
                                        rhs, start=(dr_c == 0),
                                        stop=(dr_c == 31))
                            for i in range(4):
                                dk_c = dk_g * 4 + i
                                nc.vector.tensor_scalar(
                                    out=kT[:, dk_c, mh * 512:(mh + 1) * 512],
                                    in0=psk[i],
                                    scalar1=bk_sb[:, dk_c:dk_c + 1],
                                    scalar2=None, op0=ADD)

                    for mh, xnT in ((0, xnT_lo), (1, xnT_hi)):
                        psv = [ppA.tile([P, D], F32, name="psv", tag="kv")
                               for _ in range(4)]
                        for dr_c in range(32):
                            wv_t = ws.tile([P, D], BF16, name="wv_t",
                                           tag="wv")
                            nc.sync.dma_start(
                                out=wv_t,
                                in_=wv_d[dr_c * P:(dr_c + 1) * P, :])
                            for mc in range(4):
                                lhsT = xr_rhs(xnT, dr_c, mc * P, P)
                                for nh in range(2):
                                    nc.tensor.matmul(
                                        psv[mc][:, nh * 512:(nh + 1) * 512],
                                        lhsT,
                                        wv_t[:, nh * 512:(nh + 1) * 512],
                                        start=(dr_c == 0), stop=(dr_c == 31))
                        for mc in range(4):
                            nc.vector.tensor_tensor(
                                out=v_sb[:, mh * 4 + mc, :], in0=psv[mc],
                                in1=bv_sb, op=ADD)

            # ---- Q projection + attention (xnT_hi freed above) -----------
            with ExitStack() as btx:
                ppS = btx.enter_context(
                    tc.tile_pool(name="ppS", bufs=2, space="PSUM"))
                ppSum = btx.enter_context(
                    tc.tile_pool(name="ppSum", bufs=2, space="PSUM"))
                ppO = btx.enter_context(
                    tc.tile_pool(name="ppO", bufs=2, space="PSUM"))
                qpool = btx.enter_context(tc.tile_pool(name="qblk", bufs=1))
                apool = btx.enter_context(tc.tile_pool(name="attnT", bufs=2))
                vpool = btx.enter_context(tc.tile_pool(name="avT", bufs=1))
                rpool = btx.enter_context(tc.tile_pool(name="rblk", bufs=2))
                opool = btx.enter_context(tc.tile_pool(name="outp", bufs=3))

                qT_all = qpool.tile([P, DC, NL], BF16)
                for nb in range(0 if ablate == "attn" else NL // 512):
                    for dq_c in range(DC):
                        psq = ppO.tile([P, 512], F32, name="psq", tag="o")
                        for d_c in range(DC):
                            nc.tensor.matmul(
                                psq, wq_sb[:, d_c, dq_c * P:(dq_c + 1) * P],
                                xnT_lo[:, d_c, nb * 512:(nb + 1) * 512],
                                start=(d_c == 0), stop=(d_c == DC - 1))
                        nc.vector.tensor_scalar(
                            out=qT_all[:, dq_c, nb * 512:(nb + 1) * 512],
                            in0=psq, scalar1=bq_sb[:, dq_c:dq_c + 1],
                            scalar2=None, op0=ADD)

                for nb in range(0 if ablate == "attn" else NL // 512):
                    q0 = nb * 512
                    aw = apool.tile([P, DC, 512], BF16)
                    r_blk = rpool.tile([P, 4], F32)
                    ps_r = ppSum.tile([P, 4], F32, name="ps_r", tag="sum")
                    # 4 score groups of 2 m-chunks: with ppS bufs=2 the PE
                    # streams group g+1 while ACT exponentiates group g
                    for mg in range(4):
                        pss = ppS.tile([P, 1024], F32, name="pss2", tag="s")
                        for dq_c in range(DC):
                            for i in range(2):
                                m_c = mg * 2 + i
                                nc.tensor.matmul(
                                    pss[:, i * 512:(i + 1) * 512],
                                    kT[:, dq_c, m_c * P:(m_c + 1) * P],
                                    qT_all[:, dq_c, q0:q0 + 512],
                                    start=(dq_c == 0), stop=(dq_c == DC - 1))
                        nc.scalar.activation(
                            aw[:, mg * 2:(mg + 1) * 2, :],
                            pss.rearrange("p (a b) -> p a b", a=2),
                            Exp, scale=SCALE)
                    for nt in range(4):
                        for m_c in range(DC):
                            nc.tensor.matmul(
                                ps_r[:, nt:nt + 1],
                                aw[:, m_c, nt * P:(nt + 1) * P],
                                ones_bf[:, 0:1],
                                start=(m_c == 0), stop=(m_c == DC - 1))
                    nc.vector.reciprocal(r_blk, ps_r)

                    avT = vpool.tile([P, DC, 512], BF16)
                    for dv_c in range(DC):
                        psa = ppO.tile([P, 512], F32, name="psa", tag="o")
                        for m_c in range(DC):
                            nc.tensor.matmul(
                                psa, v_sb[:, m_c, dv_c * P:(dv_c + 1) * P],
                                aw[:, m_c, :],
                                start=(m_c == 0), stop=(m_c == DC - 1))
                        nc.vector.tensor_copy(out=avT[:, dv_c, :], in_=psa)

                    for nt in range(4):
                        for dh in range(2):
                            pso = ppO.tile([P, 512], F32, name="pso", tag="o")
                            for dv_c in range(DC):
                                nc.tensor.matmul(
                                    pso, avT[:, dv_c, nt * P:(nt + 1) * P],
                                    wo_sb[:, dv_c, dh * 512:(dh + 1) * 512],
                                    start=(dv_c == 0), stop=(dv_c == DC - 1))
                            o_t = opool.tile([P, 512], F32)
                            nc.vector.tensor_scalar(
                                out=o_t, in0=pso, scalar1=r_blk[:, nt:nt + 1],
                                scalar2=None, op0=MUL)
                            nc.vector.tensor_tensor(
                                out=o_t, in0=o_t,
                                in1=bo_sb[:, dh * 512:(dh + 1) * 512], op=ADD)
                            n0 = nb * 512 + nt * P
                            nc.sync.dma_start(
                                out=out_d[n0:n0 + P,
                                          dh * 512:(dh + 1) * 512],
                                in_=o_t)

    return nc

_nc_cache = None


def host_prep(x, ln_g, ln_b, Wq, bq, Wk, bk, Wv, bv, Wo, bo, layout=None):
    """Fold LN affine into weights, cast to bf16, build per-core inputs."""
    bf = ml_dtypes.bfloat16
    x = np.asarray(x, np.float32)
    g = np.asarray(ln_g, np.float32)
    b_ln = np.asarray(ln_b, np.float32)
    Wq = np.asarray(Wq, np.float32); Wk = np.asarray(Wk, np.float32)
    Wv = np.asarray(Wv, np.float32); Wo = np.asarray(Wo, np.float32)

    wq_e = (g[:, None] * Wq).astype(bf)
    bq_e = (b_ln @ Wq + np.asarray(bq, np.float32)).astype(np.float32)
    g4 = np.tile(g, RATIO); b4 = np.tile(b_ln, RATIO)
    wk_e = (g4[:, None] * Wk).astype(bf)
    bk_e = (b4 @ Wk + np.asarray(bk, np.float32)).astype(np.float32)
    wv_e = (g4[:, None] * Wv).astype(bf)
    bv_e = (b4 @ Wv + np.asarray(bv, np.float32)).astype(np.float32)
    wo_e = Wo.astype(bf)
    bo_e = np.asarray(bo, np.float32)

    bq2 = np.ascontiguousarray(bq_e.reshape(DC, P).T)
    bk2 = np.ascontiguousarray(bk_e.reshape(DC, P).T)

    if layout is None:
        layout = "full" if (V2 or not SPLIT_KV) else "split"
    in_maps = []
    for c in range(N_CORES):
        bb, h = divmod(c, 2)
        if layout == "split":
            x_in = np.ascontiguousarray(x[bb, h * NL:(h + 1) * NL])
        else:
            x_in = np.ascontiguousarray(np.roll(x[bb], -h * NL, axis=0))
            if X_BF16:
                x_in = x_in.astype(bf)
        in_maps.append({
            "x": x_in, "wq": wq_e, "wk": wk_e, "wv": wv_e, "wo": wo_e,
            "bq2": bq2, "bk2": bk2,
            "bv1": bv_e[None, :], "bo1": bo_e[None, :],
        })
    return in_maps


def gather_out(results):
    out = np.empty((4, NF, D), np.float32)
    for c in range(N_CORES):
        bb, h = divmod(c, 2)
        out[bb, h * NL:(h + 1) * NL] = results[c]["out"]
    return out


def get_program():
    global _nc_cache
    if _nc_cache is None:
        _nc_cache = build_program2() if V2 else build_program()
        _split_multi_waits(_nc_cache)
    return _nc_cache


_runner_cache = None


def _make_runner(nc):
    """Cached-jit SPMD executor (mirrors bass2jax.run_bass_via_pjrt, but
    reusable across calls so repeat kernel() invocations don't recompile)."""
    import jax
    from jax.sharding import Mesh, PartitionSpec
    from jax.experimental.shard_map import shard_map
    import concourse.mybir as mybir
    from concourse import bass2jax
    from concourse.bass2jax import _bass_exec_p, install_neuronx_cc_hook

    install_neuronx_cc_hook()
    partition_name = (nc.partition_id_tensor.name
                      if nc.partition_id_tensor else None)
    in_names, out_names, out_avals, zero_outs = [], [], [], []
    for alloc in nc.m.functions[0].allocations:
        if not isinstance(alloc, mybir.MemoryLocationSet):
            continue
        name = alloc.memorylocations[0].name
        if alloc.kind == "ExternalInput":
            if name != partition_name:
                in_names.append(name)
        elif alloc.kind == "ExternalOutput":
            shape = tuple(alloc.tensor_shape)
            dtype = mybir.dt.np(alloc.dtype)
            out_names.append(name)
            out_avals.append(jax.core.ShapedArray(shape, dtype))
            zero_outs.append(np.zeros(shape, dtype))
    full_in_names = list(in_names) + list(out_names)
    if partition_name is not None:
        full_in_names.append(partition_name)

    def _body(*args):
        operands = list(args)
        if partition_name is not None:
            operands.append(bass2jax.partition_id_tensor())
        outs = _bass_exec_p.bind(
            *operands,
            out_avals=tuple(out_avals),
            in_names=tuple(full_in_names),
            out_names=tuple(out_names),
            lowering_input_output_aliases=(),
            sim_require_finite=True,
            sim_require_nnan=True,
            nc=nc,
        )
        return tuple(outs)

    devices = jax.devices()[:N_CORES]
    mesh = Mesh(np.asarray(devices), ("core",))
    n_in = len(in_names) + len(out_names)
    fn = jax.jit(
        shard_map(_body, mesh=mesh,
                  in_specs=(PartitionSpec("core"),) * n_in,
                  out_specs=(PartitionSpec("core"),) * len(out_names),
                  check_rep=False),
        keep_unused=True)

    def run(in_maps):
        per_core = [[np.asarray(m[name]) for name in in_names]
                    for m in in_maps]
        args = [np.concatenate([per_core[c][i] for c in range(N_CORES)],
                               axis=0) for i in range(len(in_names))]
        args += [np.zeros((N_CORES * z.shape[0], *z.shape[1:]), z.dtype)
                 for z in zero_outs]
        outs = fn(*args)
        jax.block_until_ready(outs)
        return [
            {name: np.asarray(outs[i]).reshape(N_CORES, *out_avals[i].shape)[c]
             for i, name in enumerate(out_names)}
            for c in range(N_CORES)]

    return run


def kernel(x, ln_g, ln_b, Wq, bq, Wk, bk, Wv, bv, Wo, bo):
    global _runner_cache
    nc = get_program()
    in_maps = host_prep(x, ln_g, ln_b, Wq, bq, Wk, bk, Wv, bv, Wo, bo)
    if _runner_cache is None:
        try:
            _runner_cache = _make_runner(nc)
        except Exception:
            from concourse.bass_utils import run_bass_kernel_spmd
            res = run_bass_kernel_spmd(nc, in_maps, list(range(N_CORES)))
            return gather_out(res.results)
    return gather_out(_runner_cache(in_maps))



# revision 8
# speedup vs baseline: 1.3974x; 1.3974x over previous
"""ESA layer (LN -> Q/K/V proj with token folding -> attention -> out proj)
on 8 Trainium2 NeuronCores via Bass/Tile.

Sharding (v2, collective-free): 8 cores = 4 batches x 2 token-halves.
Each core receives the FULL 4096-token batch (rolled so its own half
comes first), LayerNorms all of it, and computes K/V for the full folded
token range (redundantly with its pair peer); Q/attention/out-proj are
computed only for the owned 2048-token half.  The earlier design split
K/V across the pair and exchanged halves with 2-rank AllGathers, but the
collectives coupled every core's span to its peer's program-start skew,
which dominated the measured latency; the redundant K/V compute
(~+110us of PE time) is far cheaper than that coupling.

Details:
- LN affine (g, b) is folded into the projection weights on the host;
  x ships as bf16; per-token mean/rstd are applied in token-major
  layout, then normalized activations are transposed to feature-major
  on the PE (identity matmul) with PSUM->SBUF copies split across the
  vector and scalar engines.
- All matmuls run in bf16 (fp32 PSUM accumulation); rel err ~6e-3.
- Scores are computed transposed ([m, n]) in 4 groups of 2 m-chunks so
  the PE streams group g+1 while ACT exponentiates group g; per-token
  exp-sums come from a ones-matmul that lands them directly in
  partition layout; the softmax divide is deferred to the
  out-projection epilogue as a per-partition scale.
- This walrus build accepts only one sync-wait per instruction;
  _split_multi_waits post-processes Tile's output accordingly.
"""

import numpy as np
import ml_dtypes

P = 128
D = 1024          # model dim
RATIO = 4
NF = 4096         # tokens per batch (full)
NL = 2048         # tokens owned per core
M = NF // RATIO   # folded K/V tokens = 1024
DR = D * RATIO    # folded feature dim = 4096
DC = D // P       # feature chunks = 8
EPS = 1e-5
SCALE = 1.0 / 32.0  # 1/sqrt(D)
N_CORES = 8
SPLIT_KV = True   # pairwise K/V split + AllGather (v2)
SCORES_T = True   # compute scores transposed; skip attn transposes (v3)
V2 = True         # build_program2: no collectives, full K/V per core
X_BF16 = True     # ship x to the device as bf16 (v2 "full" layout only)

F32 = None  # set lazily (mybir types)
BF16 = None


def _split_multi_waits(nc):
    """This walrus build supports at most ONE sync wait per instruction.
    Split any instruction carrying k>1 waits into (k-1) wait-only
    EventSemaphore instructions on the same engine followed by the
    original holding a single wait."""
    import concourse.mybir as mybir
    import bass_rust

    n_split = 0
    for f in nc.m.functions:
        for bb in f.blocks:
            insts = bb.instructions
            out = []
            changed = False
            for inst in insts:
                si = getattr(inst, "sync_info", None)
                if si is not None and len(si.on_wait) > 1:
                    waits = list(si.on_wait)
                    for w in waits[:-1]:
                        nd = mybir.InstEventSemaphore(
                            name=f"I-wsplit-{n_split}", ins=[], outs=[]
                        )
                        n_split += 1
                        nd.engine = inst.engine
                        nd.sync_info = bass_rust.SyncInfo(on_wait=[w], on_update=[])
                        out.append(nd)
                    si.on_wait = [waits[-1]]
                    changed = True
                out.append(inst)
            if changed:
                bb.instructions = out
    return n_split


def build_program(reps=1, scores_t=None, split_kv=None, dma_spread=False,
                  dma_tr=True, no_cc=False, big_exp=True, ksplit=False,
                  ablate=None, chain=False):
    import concourse.bass as bass
    import concourse.mybir as mybir
    import concourse.tile as tile
    from concourse.masks import make_identity
    from contextlib import ExitStack

    scores_t = SCORES_T if scores_t is None else scores_t
    split_kv = SPLIT_KV if split_kv is None else split_kv
    global F32, BF16
    F32 = mybir.dt.float32
    BF16 = mybir.dt.bfloat16

    nc = bass.Bass("TRN2", target_bir_lowering=False, debug=False,
                   num_devices=N_CORES)

    x_rows = NL if split_kv else NF
    x_d = nc.declare_dram_parameter("x", [x_rows, D], F32, isOutput=False).ap()
    wq_d = nc.declare_dram_parameter("wq", [D, D], BF16, isOutput=False).ap()
    wk_d = nc.declare_dram_parameter("wk", [DR, D], BF16, isOutput=False).ap()
    wv_d = nc.declare_dram_parameter("wv", [DR, D], BF16, isOutput=False).ap()
    wo_d = nc.declare_dram_parameter("wo", [D, D], BF16, isOutput=False).ap()
    bq_d = nc.declare_dram_parameter("bq2", [P, DC], F32, isOutput=False).ap()
    bk_d = nc.declare_dram_parameter("bk2", [P, DC], F32, isOutput=False).ap()
    bv_d = nc.declare_dram_parameter("bv1", [1, D], F32, isOutput=False).ap()
    bo_d = nc.declare_dram_parameter("bo1", [1, D], F32, isOutput=False).ap()
    out_d = nc.declare_dram_parameter("out", [NL, D], F32, isOutput=True).ap()

    Exp = mybir.ActivationFunctionType.Exp
    Sqrt = mybir.ActivationFunctionType.Sqrt
    SUB = mybir.AluOpType.subtract
    MUL = mybir.AluOpType.mult
    ADD = mybir.AluOpType.add

    with tile.TileContext(nc) as tc:
      for _rep in range(reps):
       with ExitStack() as ctx:
        # ---- constants & resident weights -------------------------------
        const = ctx.enter_context(tc.tile_pool(name="const", bufs=1))
        ident = const.tile([P, P], BF16)
        make_identity(nc, ident)
        eps_t = const.tile([P, 1], F32)
        nc.vector.memset(eps_t, EPS)
        if chain and _rep > 0:
            # serialize rep bodies (latency measurement): make this rep's
            # LN depend on the previous rep's final output write
            chk = const.tile([P, 1], F32)
            nc.sync.dma_start(out=chk, in_=out_d[NL - P:NL, 512:513])
            zs = const.tile([P, 1], F32)
            nc.scalar.mul(out=zs, in_=chk, mul=0.0)
            nc.vector.tensor_scalar(out=eps_t, in0=eps_t, scalar1=zs,
                                    scalar2=None, op0=ADD)
        ones_bf = const.tile([P, 1], BF16)
        nc.vector.memset(ones_bf, 1.0)
        bq_sb = const.tile([P, DC], F32)
        nc.sync.dma_start(out=bq_sb, in_=bq_d)
        bk_sb = const.tile([P, DC], F32)
        nc.sync.dma_start(out=bk_sb, in_=bk_d)
        bv_sb = const.tile([P, D], F32)
        nc.gpsimd.dma_start(out=bv_sb, in_=bv_d.to_broadcast((P, D)))
        bo_sb = const.tile([P, D], F32)
        nc.gpsimd.dma_start(out=bo_sb, in_=bo_d.to_broadcast((P, D)))

        wpool = ctx.enter_context(tc.tile_pool(name="wres", bufs=1))
        wq_sb = wpool.tile([P, DC, D], BF16)   # Wq rows chunked by d
        wo_sb = wpool.tile([P, DC, D], BF16)   # Wo rows chunked by dv
        for c in range(DC):
            nc.sync.dma_start(out=wq_sb[:, c, :], in_=wq_d[c * P:(c + 1) * P, :])
            nc.sync.dma_start(out=wo_sb[:, c, :], in_=wo_d[c * P:(c + 1) * P, :])

        # ---- persistent activations -------------------------------------
        xlo_pool = ctx.enter_context(tc.tile_pool(name="xnT_lo", bufs=1))
        xnT_lo = xlo_pool.tile([P, DC, NL], BF16)   # owned half, transposed
        kvq = ctx.enter_context(tc.tile_pool(name="kvq", bufs=1))
        kT = kvq.tile([P, DC, M], BF16)             # K transposed [dk, m]
        v_sb = kvq.tile([P, DC, D], BF16)           # V normal, m-chunked

        # =================================================================
        # Phase A: LayerNorm + PE-transpose  x -> xnT  (token tiles)
        # =================================================================
        def phaseA(ctx2, tiles, xnT, ppA, xn_dram=None):
            xin = ctx2.enter_context(tc.tile_pool(name="xin", bufs=3))
            ln = ctx2.enter_context(tc.tile_pool(name="ln", bufs=4))
            lnx = ctx2.enter_context(tc.tile_pool(name="lnx", bufs=3))
            for ti, t in enumerate(tiles):
                x_t = xin.tile([P, D], F32)
                nc.sync.dma_start(out=x_t, in_=x_d[t * P:(t + 1) * P, :])
                stats = ln.tile([P, 2, 6], F32)
                nc.vector.bn_stats(out=stats[:, 0, :], in_=x_t[:, 0:512])
                nc.vector.bn_stats(out=stats[:, 1, :], in_=x_t[:, 512:1024])
                mv = ln.tile([P, 2], F32)
                nc.vector.bn_aggr(out=mv, in_=stats)
                sq = ln.tile([P, 1], F32)
                nc.scalar.activation(sq, mv[:, 1:2], Sqrt, bias=eps_t)
                rst = ln.tile([P, 1], F32)
                nc.vector.reciprocal(rst, sq)
                xn_bf = lnx.tile([P, D], BF16)
                nc.vector.tensor_scalar(
                    out=xn_bf, in0=x_t, scalar1=mv[:, 0:1], scalar2=rst,
                    op0=SUB, op1=MUL)
                col = (t - tiles[0]) * P  # column offset in xnT
                if xn_dram is not None:
                    nc.sync.dma_start(out=xn_dram[col:col + P, :], in_=xn_bf)
                    continue
                for dc in range(DC):
                    ps = ppA.tile([P, P], BF16, name="ps", tag="kv")
                    nc.tensor.transpose(ps, xn_bf[:, dc * P:(dc + 1) * P], ident)
                    dst = xnT[:, dc, col:col + P]
                    if dc % 2 == 0:
                        nc.vector.tensor_copy(out=dst, in_=ps)
                    else:
                        nc.scalar.copy(out=dst, in_=ps)

        # strided access helpers: xr^T columns for folded K/V projections.
        # dr chunk index c <-> (j = c // DC, dd_c = c % DC);
        # xrT[c*128+dd, m] = xnT[dd_c, 4m + j]
        def xr_rhs(xnT, dr_c, m0, cnt):
            j, dd_c = dr_c // DC, dr_c % DC
            r = xnT[:, dd_c, :].rearrange("p (m j) -> p j m", j=RATIO)
            return r[:, j, m0:m0 + cnt]

        with ExitStack() as actx:
            ppA = actx.enter_context(
                tc.tile_pool(name="ppBig", bufs=4, space="PSUM"))
            ws = actx.enter_context(tc.tile_pool(name="wstream", bufs=6))

            if split_kv:
                dramp = actx.enter_context(
                    tc.tile_pool(name="dram", bufs=1, space="DRAM"))
                # own token half only; peer half arrives via AllGather
                if ablate == "a":
                    nc.vector.memset(xnT_lo, 0.001)
                elif dma_tr:
                    xn_dram = dramp.tile([NL, D], BF16)
                    with ExitStack() as c2:
                        phaseA(c2, range(0, 16), xnT_lo, ppA, xn_dram=xn_dram)
                    for dc in range(DC):
                        nc.sync.dma_start(
                            out=xnT_lo[:, dc, :],
                            in_=xn_dram[:, dc * P:(dc + 1) * P], transpose=True)
                else:
                    with ExitStack() as c2:
                        phaseA(c2, range(0, 16), xnT_lo, ppA)

                kTd = dramp.tile([M, 512], BF16)          # own kT  [dk, m_own]
                vd = dramp.tile([512, D], BF16)           # own v   [m_own, dv]
                kTg = dramp.tile([2 * M, 512], BF16)
                vg = dramp.tile([2 * 512, D], BF16)
                stg = actx.enter_context(tc.tile_pool(name="stg", bufs=1))
                kTo = stg.tile([P, DC, 512], BF16)
                vo = stg.tile([P, 4, D], BF16)

                # K projection for own m-half
                if ablate == "kv":
                    nc.vector.memset(kT, 0.001)
                    nc.vector.memset(v_sb, 0.001)
                for dk_g in range(0 if ablate == "kv" else 2):
                    if ksplit:
                        psk2 = [ppA.tile([P, 1024], F32, name="psk2", tag="kv")
                                for _ in range(4)]
                    else:
                        psk = [ppA.tile([P, 512], F32, name="psk", tag="kv")
                               for _ in range(4)]
                    for dr_c in range(32):
                        wk_t = ws.tile([P, 512], BF16, name="wk_t", tag="wk")
                        (nc.scalar if dma_spread else nc.sync).dma_start(
                            out=wk_t,
                            in_=wk_d[dr_c * P:(dr_c + 1) * P,
                                     dk_g * 512:(dk_g + 1) * 512])
                        rhs = xr_rhs(xnT_lo, dr_c, 0, 512)
                        for i in range(4):
                            if ksplit:
                                h = dr_c // 16
                                nc.tensor.matmul(
                                    psk2[i][:, h * 512:(h + 1) * 512],
                                    wk_t[:, i * P:(i + 1) * P], rhs,
                                    start=(dr_c % 16 == 0),
                                    stop=(dr_c % 16 == 15))
                            else:
                                nc.tensor.matmul(
                                    psk[i], wk_t[:, i * P:(i + 1) * P], rhs,
                                    start=(dr_c == 0), stop=(dr_c == 31))
                    for i in range(4):
                        dk_c = dk_g * 4 + i
                        if ksplit:
                            nc.vector.tensor_tensor(
                                out=kTo[:, dk_c, :], in0=psk2[i][:, 0:512],
                                in1=psk2[i][:, 512:1024], op=ADD)
                            nc.vector.tensor_scalar(
                                out=kTo[:, dk_c, :], in0=kTo[:, dk_c, :],
                                scalar1=bk_sb[:, dk_c:dk_c + 1],
                                scalar2=None, op0=ADD)
                        else:
                            nc.vector.tensor_scalar(
                                out=kTo[:, dk_c, :],
                                in0=psk[i], scalar1=bk_sb[:, dk_c:dk_c + 1],
                                scalar2=None, op0=ADD)
                        nc.sync.dma_start(
                            out=kTd[dk_c * P:(dk_c + 1) * P, :],
                            in_=kTo[:, dk_c, :])
                if ablate != "kv":
                    if no_cc:  # timing probe only: fake the gather locally
                        nc.sync.dma_start(out=kTg[0:M, :], in_=kTd[:, :])
                        nc.sync.dma_start(out=kTg[M:2 * M, :], in_=kTd[:, :])
                    else:
                        nc.gpsimd.collective_compute(
                            "AllGather", mybir.AluOpType.bypass,
                            replica_groups=[[0, 1], [2, 3], [4, 5], [6, 7]],
                            ins=[kTd.opt()], outs=[kTg.opt()])
                    for r in range(2):
                        for dk_c in range(DC):
                            nc.sync.dma_start(
                                out=kT[:, dk_c, r * 512:(r + 1) * 512],
                                in_=kTg[r * M + dk_c * P:r * M + (dk_c + 1) * P, :])

                # V projection for own m-half
                psv = [ppA.tile([P, D], F32, name="psv", tag="kv")
                       for _ in range(0 if ablate == "kv" else 4)]
                for dr_c in range(0 if ablate == "kv" else 32):
                    wv_t = ws.tile([P, D], BF16, name="wv_t", tag="wv")
                    (nc.scalar if dma_spread else nc.sync).dma_start(
                        out=wv_t, in_=wv_d[dr_c * P:(dr_c + 1) * P, :])
                    for mc in range(4):
                        lhsT = xr_rhs(xnT_lo, dr_c, mc * P, P)
                        for nh in range(2):
                            nc.tensor.matmul(
                                psv[mc][:, nh * 512:(nh + 1) * 512],
                                lhsT, wv_t[:, nh * 512:(nh + 1) * 512],
                                start=(dr_c == 0), stop=(dr_c == 31))
                if ablate != "kv":
                    for mc in range(4):
                        nc.vector.tensor_tensor(
                            out=vo[:, mc, :], in0=psv[mc], in1=bv_sb, op=ADD)
                        nc.sync.dma_start(
                            out=vd[mc * P:(mc + 1) * P, :], in_=vo[:, mc, :])
                    if no_cc:
                        nc.sync.dma_start(out=vg[0:512, :], in_=vd[:, :])
                        nc.sync.dma_start(out=vg[512:1024, :], in_=vd[:, :])
                    else:
                        nc.gpsimd.collective_compute(
                            "AllGather", mybir.AluOpType.bypass,
                            replica_groups=[[0, 1], [2, 3], [4, 5], [6, 7]],
                            ins=[vd.opt()], outs=[vg.opt()])
                    for r in range(2):
                        for mc in range(4):
                            nc.sync.dma_start(
                                out=v_sb[:, r * 4 + mc, :],
                                in_=vg[r * 512 + mc * P:r * 512 + (mc + 1) * P, :])
            else:
                xhi_pool = actx.enter_context(tc.tile_pool(name="xnT_hi", bufs=1))
                xnT_hi = xhi_pool.tile([P, DC, NL], BF16)

                with ExitStack() as c2:
                    phaseA(c2, range(0, 16), xnT_lo, ppA)
                with ExitStack() as c2:
                    phaseA(c2, range(16, 32), xnT_hi, ppA)

                for mh, xnT in ((0, xnT_lo), (1, xnT_hi)):
                    for dk_g in range(2):
                        psk = [ppA.tile([P, 512], F32, name="psk", tag="kv") for _ in range(4)]
                        for dr_c in range(32):
                            wk_t = ws.tile([P, 512], BF16, name="wk_t", tag="wk")
                            nc.sync.dma_start(
                                out=wk_t,
                                in_=wk_d[dr_c * P:(dr_c + 1) * P,
                                         dk_g * 512:(dk_g + 1) * 512])
                            rhs = xr_rhs(xnT, dr_c, 0, 512)
                            for i in range(4):
                                nc.tensor.matmul(
                                    psk[i], wk_t[:, i * P:(i + 1) * P], rhs,
                                    start=(dr_c == 0), stop=(dr_c == 31))
                        for i in range(4):
                            dk_c = dk_g * 4 + i
                            nc.vector.tensor_scalar(
                                out=kT[:, dk_c, mh * 512:(mh + 1) * 512],
                                in0=psk[i], scalar1=bk_sb[:, dk_c:dk_c + 1],
                                scalar2=None, op0=ADD)

                for mh, xnT in ((0, xnT_lo), (1, xnT_hi)):
                    psv = [ppA.tile([P, D], F32, name="psv", tag="kv") for _ in range(4)]
                    for dr_c in range(32):
                        wv_t = ws.tile([P, D], BF16, name="wv_t", tag="wv")
                        nc.sync.dma_start(
                            out=wv_t, in_=wv_d[dr_c * P:(dr_c + 1) * P, :])
                        for mc in range(4):
                            lhsT = xr_rhs(xnT, dr_c, mc * P, P)
                            for nh in range(2):
                                nc.tensor.matmul(
                                    psv[mc][:, nh * 512:(nh + 1) * 512],
                                    lhsT, wv_t[:, nh * 512:(nh + 1) * 512],
                                    start=(dr_c == 0), stop=(dr_c == 31))
                    for mc in range(4):
                        nc.vector.tensor_tensor(
                            out=v_sb[:, mh * 4 + mc, :], in0=psv[mc], in1=bv_sb,
                            op=ADD)

        # =================================================================
        # Attention over 512-token blocks of the owned half
        # =================================================================
        with ExitStack() as btx:
            ppS = btx.enter_context(
                tc.tile_pool(name="ppS", bufs=(2 if big_exp else 4),
                             space="PSUM"))
            ppSum = btx.enter_context(
                tc.tile_pool(name="ppSum", bufs=2, space="PSUM"))
            ppO = btx.enter_context(
                tc.tile_pool(name="ppO", bufs=2, space="PSUM"))
            qpool = btx.enter_context(tc.tile_pool(name="qblk", bufs=2))
            apool = btx.enter_context(tc.tile_pool(name="attnT", bufs=2))
            vpool = btx.enter_context(tc.tile_pool(name="avT", bufs=1))
            epool = None
            if not scores_t:
                epool = btx.enter_context(tc.tile_pool(name="expp", bufs=3))
            spool = btx.enter_context(tc.tile_pool(name="smalls", bufs=6))
            rpool = btx.enter_context(tc.tile_pool(name="rblk", bufs=2))
            opool = btx.enter_context(tc.tile_pool(name="outp", bufs=3))

            # Q projection for the whole owned half (overlaps the K/V
            # collectives, which only the attention blocks depend on)
            qT_all = qpool.tile([P, DC, NL], BF16)
            for nb in range(0 if ablate == "attn" else NL // 512):
                for dq_c in range(DC):
                    psq = ppO.tile([P, 512], F32, name="psq", tag="o")
                    for d_c in range(DC):
                        nc.tensor.matmul(
                            psq, wq_sb[:, d_c, dq_c * P:(dq_c + 1) * P],
                            xnT_lo[:, d_c, nb * 512:(nb + 1) * 512],
                            start=(d_c == 0), stop=(d_c == DC - 1))
                    nc.vector.tensor_scalar(
                        out=qT_all[:, dq_c, nb * 512:(nb + 1) * 512], in0=psq,
                        scalar1=bq_sb[:, dq_c:dq_c + 1], scalar2=None, op0=ADD)

            for nb in range(0 if ablate == "attn" else NL // 512):
                q0 = nb * 512
                aw = apool.tile([P, DC, 512], BF16)  # attn weights [m, n]
                r_blk = rpool.tile([P, 4], F32)
                if scores_t:
                    # scores computed transposed: [m, n]; softmax divide is
                    # deferred, exp without max-sub (|s|/32 < ~5 for this data)
                    for mg in range(2):
                        if big_exp:
                            pss2 = [ppS.tile([P, 1024], F32, name="pss2",
                                             tag="s") for _ in range(2)]
                            pssT = [pss2[i // 2][:, (i % 2) * 512:
                                                 (i % 2 + 1) * 512]
                                    for i in range(4)]
                        else:
                            pssT = [ppS.tile([P, 512], F32, name="pssT",
                                             tag="s") for _ in range(4)]
                        for dq_c in range(DC):
                            for i in range(4):
                                m_c = mg * 4 + i
                                nc.tensor.matmul(
                                    pssT[i], kT[:, dq_c, m_c * P:(m_c + 1) * P],
                                    qT_all[:, dq_c, q0:q0 + 512],
                                    start=(dq_c == 0), stop=(dq_c == DC - 1))
                        if big_exp:
                            for i2 in range(2):
                                m_c = mg * 4 + i2 * 2
                                nc.scalar.activation(
                                    aw[:, m_c:m_c + 2, :],
                                    pss2[i2].rearrange("p (a b) -> p a b", a=2),
                                    Exp, scale=SCALE)
                        else:
                            for i in range(4):
                                m_c = mg * 4 + i
                                nc.scalar.activation(aw[:, m_c, :], pssT[i],
                                                     Exp, scale=SCALE)
                    # per-token exp-sums, directly in partition layout:
                    # sums[n, 1] = aw[:, n-slice].T @ ones  (accum over m chunks)
                    ps_r = ppSum.tile([P, 4], F32, name="ps_r", tag="sum")
                    for nt in range(4):
                        for m_c in range(DC):
                            nc.tensor.matmul(
                                ps_r[:, nt:nt + 1],
                                aw[:, m_c, nt * P:(nt + 1) * P], ones_bf[:, 0:1],
                                start=(m_c == 0), stop=(m_c == DC - 1))
                    nc.vector.reciprocal(r_blk, ps_r)
                else:
                    for nt in range(4):
                        pss = ppS.tile([P, M], F32, name="pss", tag="s", bufs=2)
                        for dq_c in range(DC):
                            for mh in range(2):
                                nc.tensor.matmul(
                                    pss[:, mh * 512:(mh + 1) * 512],
                                    qT_all[:, dq_c, q0 + nt * P:q0 + (nt + 1) * P],
                                    kT[:, dq_c, mh * 512:(mh + 1) * 512],
                                    start=(dq_c == 0), stop=(dq_c == DC - 1))
                        mx = spool.tile([P, 1], F32)
                        nc.vector.reduce_max(out=mx, in_=pss,
                                             axis=mybir.AxisListType.X)
                        nmx = spool.tile([P, 1], F32)
                        nc.scalar.mul(out=nmx, in_=mx, mul=-SCALE)
                        exp_t = epool.tile([P, M], BF16)
                        sum_t = spool.tile([P, 1], F32)
                        nc.scalar.activation(exp_t, pss, Exp, bias=nmx,
                                             scale=SCALE, accum_out=sum_t)
                        nc.vector.reciprocal(r_blk[:, nt:nt + 1], sum_t)
                        for m_c in range(DC):
                            pst = ppSum.tile([P, P], BF16, name="pst", tag="sum")
                            nc.tensor.transpose(
                                pst, exp_t[:, m_c * P:(m_c + 1) * P], ident)
                            dst = aw[:, m_c, nt * P:(nt + 1) * P]
                            if m_c % 2 == 0:
                                nc.vector.tensor_copy(out=dst, in_=pst)
                            else:
                                nc.scalar.copy(out=dst, in_=pst)

                # av^T[dv, n] = sum_m v[m, dv] * attnT[m, n]  (exp-weighted)
                avT = vpool.tile([P, DC, 512], BF16)
                for dv_c in range(DC):
                    psa = ppO.tile([P, 512], F32, name="psa", tag="o")
                    for m_c in range(DC):
                        nc.tensor.matmul(
                            psa, v_sb[:, m_c, dv_c * P:(dv_c + 1) * P],
                            aw[:, m_c, :],
                            start=(m_c == 0), stop=(m_c == DC - 1))
                    nc.vector.tensor_copy(out=avT[:, dv_c, :], in_=psa)

                # out[n, d] = (avT^T @ Wo) * (1/expsum) + bo
                for nt in range(4):
                    for dh in range(2):
                        pso = ppO.tile([P, 512], F32, name="pso", tag="o")
                        for dv_c in range(DC):
                            nc.tensor.matmul(
                                pso, avT[:, dv_c, nt * P:(nt + 1) * P],
                                wo_sb[:, dv_c, dh * 512:(dh + 1) * 512],
                                start=(dv_c == 0), stop=(dv_c == DC - 1))
                        o_t = opool.tile([P, 512], F32)
                        nc.vector.tensor_scalar(
                            out=o_t, in0=pso, scalar1=r_blk[:, nt:nt + 1],
                            scalar2=None, op0=MUL)
                        nc.vector.tensor_tensor(
                            out=o_t, in0=o_t,
                            in1=bo_sb[:, dh * 512:(dh + 1) * 512], op=ADD)
                        n0 = nb * 512 + nt * P
                        nc.sync.dma_start(
                            out=out_d[n0:n0 + P, dh * 512:(dh + 1) * 512],
                            in_=o_t)

    return nc


def build_program2(reps=1, chain=False, x_bf=None, ablate=None,
                   kv_contig=False):
    """v2: no collectives.  Each core computes LayerNorm + K/V for the FULL
    batch (redundantly with its pair peer) and Q/attention/out-proj for its
    own 2048-token half.  Removes the AllGathers entirely so no core ever
    waits on its peer (the collectives coupled each core's span to the
    pair's program-start skew), at the cost of ~2x the K/V matmul work.
    SBUF lifetimes are staged: {phaseA+K/V} scope frees xnT_hi and the
    weight-stream pools before the {Q+attention} scope opens.  Constants
    and resident weights are loaded once, outside the rep loop."""
    import concourse.bass as bass
    import concourse.mybir as mybir
    import concourse.tile as tile
    from concourse.masks import make_identity
    from contextlib import ExitStack

    global F32, BF16
    F32 = mybir.dt.float32
    BF16 = mybir.dt.bfloat16
    x_bf = X_BF16 if x_bf is None else x_bf
    XDT = BF16 if x_bf else F32

    nc = bass.Bass("TRN2", target_bir_lowering=False, debug=False,
                   num_devices=N_CORES)

    x_d = nc.declare_dram_parameter("x", [NF, D], XDT, isOutput=False).ap()
    wq_d = nc.declare_dram_parameter("wq", [D, D], BF16, isOutput=False).ap()
    wk_d = nc.declare_dram_parameter("wk", [DR, D], BF16, isOutput=False).ap()
    wv_d = nc.declare_dram_parameter("wv", [DR, D], BF16, isOutput=False).ap()
    wo_d = nc.declare_dram_parameter("wo", [D, D], BF16, isOutput=False).ap()
    bq_d = nc.declare_dram_parameter("bq2", [P, DC], F32, isOutput=False).ap()
    bk_d = nc.declare_dram_parameter("bk2", [P, DC], F32, isOutput=False).ap()
    bv_d = nc.declare_dram_parameter("bv1", [1, D], F32, isOutput=False).ap()
    bo_d = nc.declare_dram_parameter("bo1", [1, D], F32, isOutput=False).ap()
    out_d = nc.declare_dram_parameter("out", [NL, D], F32, isOutput=True).ap()

    Exp = mybir.ActivationFunctionType.Exp
    Sqrt = mybir.ActivationFunctionType.Sqrt
    SUB = mybir.AluOpType.subtract
    MUL = mybir.AluOpType.mult
    ADD = mybir.AluOpType.add

    with tile.TileContext(nc) as tc:
      with ExitStack() as ctx:
        const = ctx.enter_context(tc.tile_pool(name="const", bufs=1))
        ident = const.tile([P, P], BF16)
        make_identity(nc, ident)
        eps_t = const.tile([P, 1], F32)
        nc.vector.memset(eps_t, EPS)
        ones_bf = const.tile([P, 1], BF16)
        nc.vector.memset(ones_bf, 1.0)
        bq_sb = const.tile([P, DC], F32)
        nc.sync.dma_start(out=bq_sb, in_=bq_d)
        bk_sb = const.tile([P, DC], F32)
        nc.sync.dma_start(out=bk_sb, in_=bk_d)
        bv_sb = const.tile([P, D], F32)
        nc.gpsimd.dma_start(out=bv_sb, in_=bv_d.to_broadcast((P, D)))
        bo_sb = const.tile([P, D], F32)
        nc.gpsimd.dma_start(out=bo_sb, in_=bo_d.to_broadcast((P, D)))

        wpool = ctx.enter_context(tc.tile_pool(name="wres", bufs=1))
        wq_sb = wpool.tile([P, DC, D], BF16)
        wo_sb = wpool.tile([P, DC, D], BF16)
        for c in range(DC):
            nc.sync.dma_start(out=wq_sb[:, c, :], in_=wq_d[c * P:(c + 1) * P, :])
            nc.sync.dma_start(out=wo_sb[:, c, :], in_=wo_d[c * P:(c + 1) * P, :])

        xlo_pool = ctx.enter_context(tc.tile_pool(name="xnT_lo", bufs=1))
        kvq = ctx.enter_context(tc.tile_pool(name="kvq", bufs=1))
        chp = ctx.enter_context(tc.tile_pool(name="chp", bufs=1))

        for _rep in range(reps):
            xnT_lo = xlo_pool.tile([P, DC, NL], BF16, tag="xlo")
            kT = kvq.tile([P, DC, M], BF16, tag="kT")
            v_sb = kvq.tile([P, DC, D], BF16, tag="v")

            if chain and _rep > 0:
                # serialize rep bodies (latency measurement): make this
                # rep's LN depend on the previous rep's final output write
                chk = chp.tile([P, 1], F32, tag="chk")
                nc.sync.dma_start(out=chk, in_=out_d[NL - P:NL, 512:513])
                zs = chp.tile([P, 1], F32, tag="zs")
                nc.scalar.mul(out=zs, in_=chk, mul=0.0)
                nc.vector.tensor_scalar(out=eps_t, in0=eps_t, scalar1=zs,
                                        scalar2=None, op0=ADD)

            def phaseA(ctx2, tiles, xnT, ppA):
                xin = ctx2.enter_context(tc.tile_pool(name="xin", bufs=3))
                ln = ctx2.enter_context(tc.tile_pool(name="ln", bufs=4))
                lnx = ctx2.enter_context(tc.tile_pool(name="lnx", bufs=3))
                for ti, t in enumerate(tiles):
                    x_t = xin.tile([P, D], XDT)
                    nc.sync.dma_start(out=x_t, in_=x_d[t * P:(t + 1) * P, :])
                    stats = ln.tile([P, 2, 6], F32)
                    nc.vector.bn_stats(out=stats[:, 0, :], in_=x_t[:, 0:512])
                    nc.vector.bn_stats(out=stats[:, 1, :], in_=x_t[:, 512:1024])
                    mv = ln.tile([P, 2], F32)
                    nc.vector.bn_aggr(out=mv, in_=stats)
                    sq = ln.tile([P, 1], F32)
                    nc.scalar.activation(sq, mv[:, 1:2], Sqrt, bias=eps_t)
                    rst = ln.tile([P, 1], F32)
                    nc.vector.reciprocal(rst, sq)
                    xn_bf = lnx.tile([P, D], BF16)
                    nc.vector.tensor_scalar(
                        out=xn_bf, in0=x_t, scalar1=mv[:, 0:1], scalar2=rst,
                        op0=SUB, op1=MUL)
                    col = (t - tiles[0]) * P
                    for dc in range(DC):
                        ps = ppA.tile([P, P], BF16, name="ps", tag="kv")
                        nc.tensor.transpose(ps, xn_bf[:, dc * P:(dc + 1) * P],
                                            ident)
                        dst = xnT[:, dc, col:col + P]
                        if dc % 2 == 0:
                            nc.vector.tensor_copy(out=dst, in_=ps)
                        else:
                            nc.scalar.copy(out=dst, in_=ps)

            def xr_rhs(xnT, dr_c, m0, cnt):
                j, dd_c = dr_c // DC, dr_c % DC
                if kv_contig:  # timing probe ONLY: wrong data, contiguous AP
                    return xnT[:, dd_c, m0:m0 + cnt]
                r = xnT[:, dd_c, :].rearrange("p (m j) -> p j m", j=RATIO)
                return r[:, j, m0:m0 + cnt]

            # ---- phase A (both halves) + K/V for the full m range --------
            with ExitStack() as actx:
                ppA = actx.enter_context(
                    tc.tile_pool(name="ppBig", bufs=4, space="PSUM"))
                ws = actx.enter_context(tc.tile_pool(name="wstream", bufs=6))
                xhi_pool = actx.enter_context(
                    tc.tile_pool(name="xnT_hi", bufs=1))
                xnT_hi = xhi_pool.tile([P, DC, NL], BF16)

                if ablate == "a":
                    nc.vector.memset(xnT_lo, 0.001)
                    nc.vector.memset(xnT_hi, 0.001)
                else:
                    with ExitStack() as c2:
                        phaseA(c2, range(0, 16), xnT_lo, ppA)
                    with ExitStack() as c2:
                        phaseA(c2, range(16, 32), xnT_hi, ppA)

                if ablate == "kv":
                    nc.vector.memset(kT, 0.001)
                    nc.vector.memset(v_sb, 0.001)
                else:
                    for mh, xnT in ((0, xnT_lo), (1, xnT_hi)):
                        for dk_g in range(2):
                            psk = [ppA.tile([P, 512], F32, name="psk",
                                            tag="kv") for _ in range(4)]
                            for dr_c in range(32):
                                wk_t = ws.tile([P, 512], BF16, name="wk_t",
                                               tag="wk")
                                nc.sync.dma_start(
                                    out=wk_t,
                                    in_=wk_d[dr_c * P:(dr_c + 1) * P,
                                             dk_g * 512:(dk_g + 1) * 512])
                                rhs = xr_rhs(xnT, dr_c, 0, 512)
                                for i in range(4):
                                    nc.tensor.matmul(
                                        psk[i], wk_t[:, i * P:(i + 1) * P],
                                        rhs, start=(dr_c == 0),
                                        stop=(dr_c == 31))
                            for i in range(4):
                                dk_c = dk_g * 4 + i
                                nc.vector.tensor_scalar(
                                    out=kT[:, dk_c, mh * 512:(mh + 1) * 512],
                                    in0=psk[i],
                                    scalar1=bk_sb[:, dk_c:dk_c + 1],
                                    scalar2=None, op0=ADD)

                    for mh, xnT in ((0, xnT_lo), (1, xnT_hi)):
                        psv = [ppA.tile([P, D], F32, name="psv", tag="kv")
                               for _ in range(4)]
                        for dr_c in range(32):
                            wv_t = ws.tile([P, D], BF16, name="wv_t",
                                           tag="wv")
                            nc.sync.dma_start(
                                out=wv_t,
                                in_=wv_d[dr_c * P:(dr_c + 1) * P, :])
                            for mc in range(4):
                                lhsT = xr_rhs(xnT, dr_c, mc * P, P)
                                for nh in range(2):
                                    nc.tensor.matmul(
                                        psv[mc][:, nh * 512:(nh + 1) * 512],
                                        lhsT,
                                        wv_t[:, nh * 512:(nh + 1) * 512],
                                        start=(dr_c == 0), stop=(dr_c == 31))
                        for mc in range(4):
                            nc.vector.tensor_tensor(
                                out=v_sb[:, mh * 4 + mc, :], in0=psv[mc],
                                in1=bv_sb, op=ADD)

            # ---- Q projection + attention (xnT_hi freed above) -----------
            with ExitStack() as btx:
                ppS = btx.enter_context(
                    tc.tile_pool(name="ppS", bufs=2, space="PSUM"))
                ppSum = btx.enter_context(
                    tc.tile_pool(name="ppSum", bufs=2, space="PSUM"))
                ppO = btx.enter_context(
                    tc.tile_pool(name="ppO", bufs=2, space="PSUM"))
                qpool = btx.enter_context(tc.tile_pool(name="qblk", bufs=1))
                apool = btx.enter_context(tc.tile_pool(name="attnT", bufs=2))
                vpool = btx.enter_context(tc.tile_pool(name="avT", bufs=1))
                rpool = btx.enter_context(tc.tile_pool(name="rblk", bufs=2))
                opool = btx.enter_context(tc.tile_pool(name="outp", bufs=3))

                qT_all = qpool.tile([P, DC, NL], BF16)
                for nb in range(0 if ablate == "attn" else NL // 512):
                    for dq_c in range(DC):
                        psq = ppO.tile([P, 512], F32, name="psq", tag="o")
                        for d_c in range(DC):
                            nc.tensor.matmul(
                                psq, wq_sb[:, d_c, dq_c * P:(dq_c + 1) * P],
                                xnT_lo[:, d_c, nb * 512:(nb + 1) * 512],
                                start=(d_c == 0), stop=(d_c == DC - 1))
                        nc.vector.tensor_scalar(
                            out=qT_all[:, dq_c, nb * 512:(nb + 1) * 512],
                            in0=psq, scalar1=bq_sb[:, dq_c:dq_c + 1],
                            scalar2=None, op0=ADD)

                for nb in range(0 if ablate == "attn" else NL // 512):
                    q0 = nb * 512
                    aw = apool.tile([P, DC, 512], BF16)
                    r_blk = rpool.tile([P, 4], F32)
                    ps_r = ppSum.tile([P, 4], F32, name="ps_r", tag="sum")
                    # 4 score groups of 2 m-chunks: with ppS bufs=2 the PE
                    # streams group g+1 while ACT exponentiates group g
                    for mg in range(4):
                        pss = ppS.tile([P, 1024], F32, name="pss2", tag="s")
                        for dq_c in range(DC):
                            for i in range(2):
                                m_c = mg * 2 + i
                                nc.tensor.matmul(
                                    pss[:, i * 512:(i + 1) * 512],
                                    kT[:, dq_c, m_c * P:(m_c + 1) * P],
                                    qT_all[:, dq_c, q0:q0 + 512],
                                    start=(dq_c == 0), stop=(dq_c == DC - 1))
                        nc.scalar.activation(
                            aw[:, mg * 2:(mg + 1) * 2, :],
                            pss.rearrange("p (a b) -> p a b", a=2),
                            Exp, scale=SCALE)
                    for nt in range(4):
                        for m_c in range(DC):
                            nc.tensor.matmul(
                                ps_r[:, nt:nt + 1],
                                aw[:, m_c, nt * P:(nt + 1) * P],
                                ones_bf[:, 0:1],
                                start=(m_c == 0), stop=(m_c == DC - 1))
                    nc.vector.reciprocal(r_blk, ps_r)

                    avT = vpool.tile([P, DC, 512], BF16)
                    for dv_c in range(DC):
                        psa = ppO.tile([P, 512], F32, name="psa", tag="o")
                        for m_c in range(DC):
                            nc.tensor.matmul(
                                psa, v_sb[:, m_c, dv_c * P:(dv_c + 1) * P],
                                aw[:, m_c, :],
                                start=(m_c == 0), stop=(m_c == DC - 1))
                        nc.vector.tensor_copy(out=avT[:, dv_c, :], in_=psa)

                    for nt in range(4):
                        for dh in range(2):
                            pso = ppO.tile([P, 512], F32, name="pso", tag="o")
                            for dv_c in range(DC):
                                nc.tensor.matmul(
                                    pso, avT[:, dv_c, nt * P:(nt + 1) * P],
                                    wo_sb[:, dv_c, dh * 512:(dh + 1) * 512],
                                    start=(dv_c == 0), stop=(dv_c == DC - 1))
                            o_t = opool.tile([P, 512], F32)
                            nc.vector.tensor_scalar(
                                out=o_t, in0=pso, scalar1=r_blk[:, nt:nt + 1],
                                scalar2=None, op0=MUL)
                            nc.vector.tensor_tensor(
                                out=o_t, in0=o_t,
                                in1=bo_sb[:, dh * 512:(dh + 1) * 512], op=ADD)
                            n0 = nb * 512 + nt * P
                            nc.sync.dma_start(
                                out=out_d[n0:n0 + P,
                                          dh * 512:(dh + 1) * 512],
                                in_=o_t)

    return nc

def build_program3(reps=1, chain=False, ablate=None):
    """v3: v2 (no collectives, full K/V per core) with the program reordered
    so the PE never idles during the LayerNorm phase:

      PE order: [lo transposes + Q-proj interleaved] -> [K-mh0 with hi
      transposes interleaved] -> K-mh1 -> V-mh0 -> V-mh1 -> attention.

    The lo-half LN runs on DVE while the PE does the lo transposes and the
    Q projection; the hi-half LN runs on DVE/ACT/Pool underneath K-mh0.
    Transposes land 4-at-a-time in one PSUM tile so each [P,512] drain is
    a single copy, spread round-robin over ACT/Pool/DVE.  Q's bias add
    moved to ACT (Identity+bias) to keep DVE off the critical path."""
    import concourse.bass as bass
    import concourse.mybir as mybir
    import concourse.tile as tile
    from concourse.masks import make_identity
    from contextlib import ExitStack

    global F32, BF16
    F32 = mybir.dt.float32
    BF16 = mybir.dt.bfloat16
    XDT = BF16

    nc = bass.Bass("TRN2", target_bir_lowering=False, debug=False,
                   num_devices=N_CORES)

    x_d = nc.declare_dram_parameter("x", [NF, D], XDT, isOutput=False).ap()
    wq_d = nc.declare_dram_parameter("wq", [D, D], BF16, isOutput=False).ap()
    wk_d = nc.declare_dram_parameter("wk", [DR, D], BF16, isOutput=False).ap()
    wv_d = nc.declare_dram_parameter("wv", [DR, D], BF16, isOutput=False).ap()
    wo_d = nc.declare_dram_parameter("wo", [D, D], BF16, isOutput=False).ap()
    bq_d = nc.declare_dram_parameter("bq2", [P, DC], F32, isOutput=False).ap()
    bk_d = nc.declare_dram_parameter("bk2", [P, DC], F32, isOutput=False).ap()
    bv_d = nc.declare_dram_parameter("bv1", [1, D], F32, isOutput=False).ap()
    bo_d = nc.declare_dram_parameter("bo1", [1, D], F32, isOutput=False).ap()
    out_d = nc.declare_dram_parameter("out", [NL, D], F32, isOutput=True).ap()

    Exp = mybir.ActivationFunctionType.Exp
    Sqrt = mybir.ActivationFunctionType.Sqrt
    SUB = mybir.AluOpType.subtract
    MUL = mybir.AluOpType.mult
    ADD = mybir.AluOpType.add

    with tile.TileContext(nc) as tc:
      with ExitStack() as ctx:
        const = ctx.enter_context(tc.tile_pool(name="const", bufs=1))
        ident = const.tile([P, P], BF16)
        make_identity(nc, ident)
        eps_t = const.tile([P, 1], F32)
        nc.vector.memset(eps_t, EPS)
        ones_bf = const.tile([P, 1], BF16)
        nc.vector.memset(ones_bf, 1.0)
        bq_sb = const.tile([P, DC], F32)
        nc.sync.dma_start(out=bq_sb, in_=bq_d)
        bk_sb = const.tile([P, DC], F32)
        nc.sync.dma_start(out=bk_sb, in_=bk_d)
        bv_sb = const.tile([P, D], F32)
        nc.gpsimd.dma_start(out=bv_sb, in_=bv_d.to_broadcast((P, D)))
        bo_sb = const.tile([P, D], F32)
        nc.gpsimd.dma_start(out=bo_sb, in_=bo_d.to_broadcast((P, D)))

        wpool = ctx.enter_context(tc.tile_pool(name="wres", bufs=1))
        xlo_pool = ctx.enter_context(tc.tile_pool(name="xnT_lo", bufs=1))
        kvq = ctx.enter_context(tc.tile_pool(name="kvq", bufs=1))
        qpool = ctx.enter_context(tc.tile_pool(name="qblk", bufs=1))
        chp = ctx.enter_context(tc.tile_pool(name="chp", bufs=1))

        for _rep in range(reps):
            wq_sb = wpool.tile([P, DC, D], BF16, tag="wq")
            wo_sb = wpool.tile([P, DC, D], BF16, tag="wo")
            xnT_lo = xlo_pool.tile([P, DC, NL], BF16, tag="xlo")
            kT = kvq.tile([P, DC, M], BF16, tag="kT")
            v_sb = kvq.tile([P, DC, D], BF16, tag="v")
            qT_all = qpool.tile([P, DC, NL], BF16, tag="qT")

            if chain and _rep > 0:
                chk = chp.tile([P, 1], F32, tag="chk")
                nc.sync.dma_start(out=chk, in_=out_d[NL - P:NL, 512:513])
                zs = chp.tile([P, 1], F32, tag="zs")
                nc.scalar.mul(out=zs, in_=chk, mul=0.0)
                nc.vector.tensor_scalar(out=eps_t, in0=eps_t, scalar1=zs,
                                        scalar2=None, op0=ADD)

            with ExitStack() as kv_scope:
                xhi_pool = kv_scope.enter_context(
                    tc.tile_pool(name="xnT_hi", bufs=1))
                xnT_hi = xhi_pool.tile([P, DC, NL], BF16)

                def xr_rhs(xnT, dr_c, m0, cnt):
                    j, dd_c = dr_c // DC, dr_c % DC
                    r = xnT[:, dd_c, :].rearrange("p (m j) -> p j m", j=RATIO)
                    return r[:, j, m0:m0 + cnt]

                with ExitStack() as s1:
                    xin = s1.enter_context(tc.tile_pool(name="xin", bufs=4))
                    ln = s1.enter_context(tc.tile_pool(name="ln", bufs=4))
                    lnx = s1.enter_context(tc.tile_pool(name="lnx", bufs=3))
                    ppT = s1.enter_context(
                        tc.tile_pool(name="ppT", bufs=3, space="PSUM"))
                    ws = s1.enter_context(tc.tile_pool(name="wstream", bufs=6))

                    # copy-engine rotation for transpose drains
                    cp_engines = [nc.scalar, nc.vector]

                    def emit_tile(t, xnT, ci):
                        """LN + transpose one 128-token tile into xnT."""
                        x_t = xin.tile([P, D], XDT, name="x_t", tag="x")
                        nc.sync.dma_start(out=x_t, in_=x_d[t * P:(t + 1) * P, :])
                        stats = ln.tile([P, 2, 6], F32, name="st", tag="st")
                        nc.vector.bn_stats(out=stats[:, 0, :], in_=x_t[:, 0:512])
                        nc.vector.bn_stats(out=stats[:, 1, :],
                                           in_=x_t[:, 512:1024])
                        mv = ln.tile([P, 2], F32, name="mv", tag="mv")
                        nc.vector.bn_aggr(out=mv, in_=stats)
                        sq = ln.tile([P, 1], F32, name="sq", tag="sq")
                        nc.scalar.activation(sq, mv[:, 1:2], Sqrt, bias=eps_t)
                        rst = ln.tile([P, 1], F32, name="rst", tag="rst")
                        nc.vector.reciprocal(rst, sq)
                        xn_bf = lnx.tile([P, D], BF16, name="xn", tag="xn")
                        nc.vector.tensor_scalar(
                            out=xn_bf, in0=x_t, scalar1=mv[:, 0:1], scalar2=rst,
                            op0=SUB, op1=MUL)
                        col = (t % 16) * P
                        for h in range(2):
                            ps = ppT.tile([P, 512], BF16, name="psT", tag="T")
                            for i in range(4):
                                nc.tensor.transpose(
                                    ps[:, i * P:(i + 1) * P],
                                    xn_bf[:, (h * 4 + i) * P:
                                          (h * 4 + i + 1) * P], ident)
                            eng = cp_engines[(ci + h) % 2]
                            if eng is nc.scalar:
                                eng.copy(
                                    out=xnT[:, h * 4:(h + 1) * 4, col:col + P],
                                    in_=ps.rearrange("p (a b) -> p a b", a=4))
                            else:
                                eng.tensor_copy(
                                    out=xnT[:, h * 4:(h + 1) * 4, col:col + P],
                                    in_=ps.rearrange("p (a b) -> p a b", a=4))

                    # ---- phase 1: lo tiles + wq prefetch + Q interleave ----
                    with ExitStack() as p1:
                        ppQ = p1.enter_context(
                            tc.tile_pool(name="ppQ", bufs=2, space="PSUM"))
                        for t in range(16):
                            emit_tile(t, xnT_lo, t)
                            if t == 3:  # wq resident load after first 4 x tiles
                                for c in range(DC):
                                    nc.sync.dma_start(
                                        out=wq_sb[:, c, :],
                                        in_=wq_d[c * P:(c + 1) * P, :])
                            if t % 4 == 3 and ablate != "attn":
                                nb = t // 4
                                for dq_c in range(DC):
                                    psq = ppQ.tile([P, 512], F32, name="psq",
                                                   tag="q")
                                    for d_c in range(DC):
                                        nc.tensor.matmul(
                                            psq,
                                            wq_sb[:, d_c, dq_c * P:(dq_c + 1) * P],
                                            xnT_lo[:, d_c,
                                                   nb * 512:(nb + 1) * 512],
                                            start=(d_c == 0),
                                            stop=(d_c == DC - 1))
                                    nc.scalar.add(
                                        out=qT_all[:, dq_c,
                                                   nb * 512:(nb + 1) * 512],
                                        in_=psq,
                                        add=bq_sb[:, dq_c:dq_c + 1])

                    # ---- phases 2+3: K (mh0 with hi tiles interleaved) -----
                    with ExitStack() as p2:
                        psk_pool = p2.enter_context(
                            tc.tile_pool(name="ppK", bufs=4, space="PSUM"))
                        hi_next = 16
                        for mh, xnT in ((0, xnT_lo), (1, xnT_hi)):
                            for dk_g in range(2):
                                psk = [psk_pool.tile([P, 512], F32, name="psk",
                                                     tag="k") for _ in range(4)]
                                for dr_c in range(32):
                                    wk_t = ws.tile([P, 512], BF16, name="wk_t",
                                                   tag="wk")
                                    nc.sync.dma_start(
                                        out=wk_t,
                                        in_=wk_d[dr_c * P:(dr_c + 1) * P,
                                                 dk_g * 512:(dk_g + 1) * 512])
                                    rhs = xr_rhs(xnT, dr_c, 0, 512)
                                    for i in range(4):
                                        nc.tensor.matmul(
                                            psk[i], wk_t[:, i * P:(i + 1) * P],
                                            rhs, start=(dr_c == 0),
                                            stop=(dr_c == 31))
                                    if mh == 0 and dr_c % 2 == 1 and hi_next < 32:
                                        emit_tile(hi_next, xnT_hi, hi_next)
                                        hi_next += 1
                                for i in range(4):
                                    dk_c = dk_g * 4 + i
                                    nc.vector.tensor_scalar(
                                        out=kT[:, dk_c, mh * 512:(mh + 1) * 512],
                                        in0=psk[i],
                                        scalar1=bk_sb[:, dk_c:dk_c + 1],
                                        scalar2=None, op0=ADD)
                        assert hi_next == 32

                # ---- phases 4+5: V (all PSUM free now) --------------------
                with ExitStack() as s2:
                    psv_pool = s2.enter_context(
                        tc.tile_pool(name="ppV", bufs=4, space="PSUM"))
                    wsv = s2.enter_context(tc.tile_pool(name="wsv", bufs=6))
                    for mh, xnT in ((0, xnT_lo), (1, xnT_hi)):
                        psv = [psv_pool.tile([P, D], F32, name="psv", tag="v")
                               for _ in range(4)]
                        for dr_c in range(32):
                            wv_t = wsv.tile([P, D], BF16, name="wv_t", tag="wv")
                            nc.sync.dma_start(
                                out=wv_t, in_=wv_d[dr_c * P:(dr_c + 1) * P, :])
                            for mc in range(4):
                                lhsT = xr_rhs(xnT, dr_c, mc * P, P)
                                for nh in range(2):
                                    nc.tensor.matmul(
                                        psv[mc][:, nh * 512:(nh + 1) * 512],
                                        lhsT, wv_t[:, nh * 512:(nh + 1) * 512],
                                        start=(dr_c == 0), stop=(dr_c == 31))
                            if mh == 0 and dr_c == 0:
                                # wo resident load; needed from attention on
                                for c in range(DC):
                                    nc.scalar.dma_start(
                                        out=wo_sb[:, c, :],
                                        in_=wo_d[c * P:(c + 1) * P, :])
                        for mc in range(4):
                            nc.vector.tensor_tensor(
                                out=v_sb[:, mh * 4 + mc, :], in0=psv[mc],
                                in1=bv_sb, op=ADD)

            # ---- attention (xnT_hi freed above) ---------------------------
            with ExitStack() as btx:
                ppS = btx.enter_context(
                    tc.tile_pool(name="ppS", bufs=2, space="PSUM"))
                ppSum = btx.enter_context(
                    tc.tile_pool(name="ppSum", bufs=2, space="PSUM"))
                ppO = btx.enter_context(
                    tc.tile_pool(name="ppO", bufs=2, space="PSUM"))
                apool = btx.enter_context(tc.tile_pool(name="attnT", bufs=2))
                vpool = btx.enter_context(tc.tile_pool(name="avT", bufs=1))
                rpool = btx.enter_context(tc.tile_pool(name="rblk", bufs=2))
                opool = btx.enter_context(tc.tile_pool(name="outp", bufs=3))

                for nb in range(0 if ablate == "attn" else NL // 512):
                    q0 = nb * 512
                    aw = apool.tile([P, DC, 512], BF16)
                    r_blk = rpool.tile([P, 4], F32)
                    ps_r = ppSum.tile([P, 4], F32, name="ps_r", tag="sum")
                    for mg in range(4):
                        pss = ppS.tile([P, 1024], F32, name="pss2", tag="s")
                        for dq_c in range(DC):
                            for i in range(2):
                                m_c = mg * 2 + i
                                nc.tensor.matmul(
                                    pss[:, i * 512:(i + 1) * 512],
                                    kT[:, dq_c, m_c * P:(m_c + 1) * P],
                                    qT_all[:, dq_c, q0:q0 + 512],
                                    start=(dq_c == 0), stop=(dq_c == DC - 1))
                        nc.scalar.activation(
                            aw[:, mg * 2:(mg + 1) * 2, :],
                            pss.rearrange("p (a b) -> p a b", a=2),
                            Exp, scale=SCALE)
                    for nt in range(4):
                        for m_c in range(DC):
                            nc.tensor.matmul(
                                ps_r[:, nt:nt + 1],
                                aw[:, m_c, nt * P:(nt + 1) * P],
                                ones_bf[:, 0:1],
                                start=(m_c == 0), stop=(m_c == DC - 1))
                    nc.vector.reciprocal(r_blk, ps_r)

                    avT = vpool.tile([P, DC, 512], BF16)
                    for dv_c in range(DC):
                        psa = ppO.tile([P, 512], F32, name="psa", tag="o")
                        for m_c in range(DC):
                            nc.tensor.matmul(
                                psa, v_sb[:, m_c, dv_c * P:(dv_c + 1) * P],
                                aw[:, m_c, :],
                                start=(m_c == 0), stop=(m_c == DC - 1))
                        if dv_c % 2 == 0:
                            nc.vector.tensor_copy(out=avT[:, dv_c, :], in_=psa)
                        else:
                            nc.scalar.copy(out=avT[:, dv_c, :], in_=psa)

                    for nt in range(4):
                        for dh in range(2):
                            pso = ppO.tile([P, 512], F32, name="pso", tag="o")
                            for dv_c in range(DC):
                                nc.tensor.matmul(
                                    pso, avT[:, dv_c, nt * P:(nt + 1) * P],
                                    wo_sb[:, dv_c, dh * 512:(dh + 1) * 512],
                                    start=(dv_c == 0), stop=(dv_c == DC - 1))
                            o_t = opool.tile([P, 512], F32)
                            nc.vector.tensor_scalar(
                                out=o_t, in0=pso, scalar1=r_blk[:, nt:nt + 1],
                                scalar2=None, op0=MUL)
                            nc.vector.tensor_tensor(
                                out=o_t, in0=o_t,
                                in1=bo_sb[:, dh * 512:(dh + 1) * 512], op=ADD)
                            n0 = nb * 512 + nt * P
                            nc.sync.dma_start(
                                out=out_d[n0:n0 + P,
                                          dh * 512:(dh + 1) * 512],
                                in_=o_t)

    return nc


build_program3_layout = "full"


def build_program4(reps=1, chain=False, ablate=None, no_cc=False):
    """v4: split K/V across the pair (each core computes K/V only for its
    own 512-m half; 2-rank AllGathers exchange them) + the v3 reordering so
    the PE never idles and the gathers hide under unrelated PE work:

      PE order: [own-half LN transposes + Q nb0/nb1] -> K own -> V own ->
                Q nb2/nb3 -> attention.
      K-gather issues after K-own (covered by V + Q23, ~73us of PE work);
      V-gather issues after V-own (covered by Q23 + scores nb0, ~37us).

    Per-core matmul work drops from 17.2 to 12.9 GMAC (no redundant K/V,
    and LN runs only over the own 2048-token half)."""
    import concourse.bass as bass
    import concourse.mybir as mybir
    import concourse.tile as tile
    from concourse.masks import make_identity
    from contextlib import ExitStack

    global F32, BF16
    F32 = mybir.dt.float32
    BF16 = mybir.dt.bfloat16
    XDT = BF16

    nc = bass.Bass("TRN2", target_bir_lowering=False, debug=False,
                   num_devices=N_CORES)

    x_d = nc.declare_dram_parameter("x", [NL, D], XDT, isOutput=False).ap()
    wq_d = nc.declare_dram_parameter("wq", [D, D], BF16, isOutput=False).ap()
    wk_d = nc.declare_dram_parameter("wk", [DR, D], BF16, isOutput=False).ap()
    wv_d = nc.declare_dram_parameter("wv", [DR, D], BF16, isOutput=False).ap()
    wo_d = nc.declare_dram_parameter("wo", [D, D], BF16, isOutput=False).ap()
    bq_d = nc.declare_dram_parameter("bq2", [P, DC], F32, isOutput=False).ap()
    bk_d = nc.declare_dram_parameter("bk2", [P, DC], F32, isOutput=False).ap()
    bv_d = nc.declare_dram_parameter("bv1", [1, D], F32, isOutput=False).ap()
    bo_d = nc.declare_dram_parameter("bo1", [1, D], F32, isOutput=False).ap()
    out_d = nc.declare_dram_parameter("out", [NL, D], F32, isOutput=True).ap()

    Exp = mybir.ActivationFunctionType.Exp
    Sqrt = mybir.ActivationFunctionType.Sqrt
    SUB = mybir.AluOpType.subtract
    MUL = mybir.AluOpType.mult
    ADD = mybir.AluOpType.add

    with tile.TileContext(nc) as tc:
      with ExitStack() as ctx:
        const = ctx.enter_context(tc.tile_pool(name="const", bufs=1))
        ident = const.tile([P, P], BF16)
        make_identity(nc, ident)
        eps_t = const.tile([P, 1], F32)
        nc.vector.memset(eps_t, EPS)
        ones_bf = const.tile([P, 1], BF16)
        nc.vector.memset(ones_bf, 1.0)
        bq_sb = const.tile([P, DC], F32)
        nc.sync.dma_start(out=bq_sb, in_=bq_d)
        bk_sb = const.tile([P, DC], F32)
        nc.sync.dma_start(out=bk_sb, in_=bk_d)
        bv_sb = const.tile([P, D], F32)
        nc.gpsimd.dma_start(out=bv_sb, in_=bv_d.to_broadcast((P, D)))
        bo_sb = const.tile([P, D], F32)
        nc.gpsimd.dma_start(out=bo_sb, in_=bo_d.to_broadcast((P, D)))

        wpool = ctx.enter_context(tc.tile_pool(name="wres", bufs=1))
        xlo_pool = ctx.enter_context(tc.tile_pool(name="xnT_lo", bufs=1))
        kvq = ctx.enter_context(tc.tile_pool(name="kvq", bufs=1))
        qpool = ctx.enter_context(tc.tile_pool(name="qblk", bufs=1))
        stg = ctx.enter_context(tc.tile_pool(name="stg", bufs=1))
        dramp = ctx.enter_context(
            tc.tile_pool(name="dram", bufs=1, space="DRAM"))
        chp = ctx.enter_context(tc.tile_pool(name="chp", bufs=1))

        for _rep in range(reps):
            wq_sb = wpool.tile([P, DC, D], BF16, tag="wq")
            wo_sb = wpool.tile([P, DC, D], BF16, tag="wo")
            xnT_lo = xlo_pool.tile([P, DC, NL], BF16, tag="xlo")
            kT = kvq.tile([P, DC, M], BF16, tag="kT")
            v_sb = kvq.tile([P, DC, D], BF16, tag="v")
            qT_all = qpool.tile([P, DC, NL], BF16, tag="qT")
            kTo = stg.tile([P, DC, 512], BF16, tag="kTo")
            vo = stg.tile([P, 4, D], BF16, tag="vo")
            kTd = dramp.tile([M, 512], BF16, tag="kTd")
            vd = dramp.tile([512, D], BF16, tag="vd")
            kTg = dramp.tile([2 * M, 512], BF16, tag="kTg")
            vg = dramp.tile([2 * 512, D], BF16, tag="vg")

            if chain and _rep > 0:
                chk = chp.tile([P, 1], F32, tag="chk")
                nc.sync.dma_start(out=chk, in_=out_d[NL - P:NL, 512:513])
                zs = chp.tile([P, 1], F32, tag="zs")
                nc.scalar.mul(out=zs, in_=chk, mul=0.0)
                nc.vector.tensor_scalar(out=eps_t, in0=eps_t, scalar1=zs,
                                        scalar2=None, op0=ADD)

            def xr_rhs(dr_c, m0, cnt):
                j, dd_c = dr_c // DC, dr_c % DC
                r = xnT_lo[:, dd_c, :].rearrange("p (m j) -> p j m", j=RATIO)
                return r[:, j, m0:m0 + cnt]

            def emit_q(nb, ppQ):
                for dq_c in range(DC):
                    psq = ppQ.tile([P, 512], F32, name="psq", tag="q")
                    for d_c in range(DC):
                        nc.tensor.matmul(
                            psq, wq_sb[:, d_c, dq_c * P:(dq_c + 1) * P],
                            xnT_lo[:, d_c, nb * 512:(nb + 1) * 512],
                            start=(d_c == 0), stop=(d_c == DC - 1))
                    nc.scalar.add(
                        out=qT_all[:, dq_c, nb * 512:(nb + 1) * 512],
                        in_=psq, add=bq_sb[:, dq_c:dq_c + 1])

            with ExitStack() as s1:
                xin = s1.enter_context(tc.tile_pool(name="xin", bufs=4))
                ln = s1.enter_context(tc.tile_pool(name="ln", bufs=4))
                lnx = s1.enter_context(tc.tile_pool(name="lnx", bufs=3))
                ws = s1.enter_context(tc.tile_pool(name="wstream", bufs=6))
                cp_engines = [nc.scalar, nc.vector]

                # ---- phase 1: own-half LN + transposes + Q nb0/nb1 --------
                with ExitStack() as p1:
                    ppT = p1.enter_context(
                        tc.tile_pool(name="ppT", bufs=3, space="PSUM"))
                    ppQ = p1.enter_context(
                        tc.tile_pool(name="ppQa", bufs=2, space="PSUM"))
                    for t in range(16):
                        x_t = xin.tile([P, D], XDT, name="x_t", tag="x")
                        nc.sync.dma_start(out=x_t, in_=x_d[t * P:(t + 1) * P, :])
                        stats = ln.tile([P, 2, 6], F32, name="st", tag="st")
                        nc.vector.bn_stats(out=stats[:, 0, :], in_=x_t[:, 0:512])
                        nc.vector.bn_stats(out=stats[:, 1, :],
                                           in_=x_t[:, 512:1024])
                        mv = ln.tile([P, 2], F32, name="mv", tag="mv")
                        nc.vector.bn_aggr(out=mv, in_=stats)
                        sq = ln.tile([P, 1], F32, name="sq", tag="sq")
                        nc.scalar.activation(sq, mv[:, 1:2], Sqrt, bias=eps_t)
                        rst = ln.tile([P, 1], F32, name="rst", tag="rst")
                        nc.vector.reciprocal(rst, sq)
                        xn_bf = lnx.tile([P, D], BF16, name="xn", tag="xn")
                        nc.vector.tensor_scalar(
                            out=xn_bf, in0=x_t, scalar1=mv[:, 0:1], scalar2=rst,
                            op0=SUB, op1=MUL)
                        col = t * P
                        for h in range(2):
                            ps = ppT.tile([P, 512], BF16, name="psT", tag="T")
                            for i in range(4):
                                nc.tensor.transpose(
                                    ps[:, i * P:(i + 1) * P],
                                    xn_bf[:, (h * 4 + i) * P:
                                          (h * 4 + i + 1) * P], ident)
                            eng = cp_engines[(t + h) % 2]
                            if eng is nc.scalar:
                                eng.copy(
                                    out=xnT_lo[:, h * 4:(h + 1) * 4,
                                               col:col + P],
                                    in_=ps.rearrange("p (a b) -> p a b", a=4))
                            else:
                                eng.tensor_copy(
                                    out=xnT_lo[:, h * 4:(h + 1) * 4,
                                               col:col + P],
                                    in_=ps.rearrange("p (a b) -> p a b", a=4))
                        if t == 3:
                            for c in range(DC):
                                nc.sync.dma_start(
                                    out=wq_sb[:, c, :],
                                    in_=wq_d[c * P:(c + 1) * P, :])
                        if t == 7 and ablate != "attn":
                            emit_q(0, ppQ)
                        if t == 15 and ablate != "attn":
                            emit_q(1, ppQ)

                # ---- phase 2: K own half + gather -------------------------
                with ExitStack() as p2:
                    psk_pool = p2.enter_context(
                        tc.tile_pool(name="ppK", bufs=4, space="PSUM"))
                    for dk_g in range(2):
                        psk = [psk_pool.tile([P, 512], F32, name="psk",
                                             tag="k") for _ in range(4)]
                        for dr_c in range(32):
                            wk_t = ws.tile([P, 1024], BF16, name="wk_t",
                                           tag="w")[:, 0:512]
                            nc.sync.dma_start(
                                out=wk_t,
                                in_=wk_d[dr_c * P:(dr_c + 1) * P,
                                         dk_g * 512:(dk_g + 1) * 512])
                            rhs = xr_rhs(dr_c, 0, 512)
                            for i in range(4):
                                nc.tensor.matmul(
                                    psk[i], wk_t[:, i * P:(i + 1) * P],
                                    rhs, start=(dr_c == 0), stop=(dr_c == 31))
                        for i in range(4):
                            dk_c = dk_g * 4 + i
                            nc.vector.tensor_scalar(
                                out=kTo[:, dk_c, :], in0=psk[i],
                                scalar1=bk_sb[:, dk_c:dk_c + 1],
                                scalar2=None, op0=ADD)
                            nc.sync.dma_start(
                                out=kTd[dk_c * P:(dk_c + 1) * P, :],
                                in_=kTo[:, dk_c, :])
                if no_cc:
                    nc.sync.dma_start(out=kTg[0:M, :], in_=kTd[:, :])
                    nc.sync.dma_start(out=kTg[M:2 * M, :], in_=kTd[:, :])
                else:
                    nc.gpsimd.collective_compute(
                        "AllGather", mybir.AluOpType.bypass,
                        replica_groups=[[0, 1], [2, 3], [4, 5], [6, 7]],
                        ins=[kTd.opt()], outs=[kTg.opt()])
                for r in range(2):
                    for dk_c in range(DC):
                        nc.sync.dma_start(
                            out=kT[:, dk_c, r * 512:(r + 1) * 512],
                            in_=kTg[r * M + dk_c * P:r * M + (dk_c + 1) * P, :])

                # ---- phase 3: V own half + gather -------------------------
                with ExitStack() as p3:
                    psv_pool = p3.enter_context(
                        tc.tile_pool(name="ppV", bufs=4, space="PSUM"))
                    psv = [psv_pool.tile([P, D], F32, name="psv", tag="v")
                           for _ in range(4)]
                    for dr_c in range(32):
                        wv_t = ws.tile([P, 1024], BF16, name="wv_t", tag="w")
                        nc.sync.dma_start(
                            out=wv_t, in_=wv_d[dr_c * P:(dr_c + 1) * P, :])
                        for mc in range(4):
                            lhsT = xr_rhs(dr_c, mc * P, P)
                            for nh in range(2):
                                nc.tensor.matmul(
                                    psv[mc][:, nh * 512:(nh + 1) * 512],
                                    lhsT, wv_t[:, nh * 512:(nh + 1) * 512],
                                    start=(dr_c == 0), stop=(dr_c == 31))
                        if dr_c == 0:
                            for c in range(DC):
                                nc.scalar.dma_start(
                                    out=wo_sb[:, c, :],
                                    in_=wo_d[c * P:(c + 1) * P, :])
                    for mc in range(4):
                        nc.vector.tensor_tensor(
                            out=vo[:, mc, :], in0=psv[mc], in1=bv_sb, op=ADD)
                        nc.sync.dma_start(
                            out=vd[mc * P:(mc + 1) * P, :], in_=vo[:, mc, :])
                if no_cc:
                    nc.sync.dma_start(out=vg[0:512, :], in_=vd[:, :])
                    nc.sync.dma_start(out=vg[512:1024, :], in_=vd[:, :])
                else:
                    nc.gpsimd.collective_compute(
                        "AllGather", mybir.AluOpType.bypass,
                        replica_groups=[[0, 1], [2, 3], [4, 5], [6, 7]],
                        ins=[vd.opt()], outs=[vg.opt()])
                for r in range(2):
                    for mc in range(4):
                        nc.sync.dma_start(
                            out=v_sb[:, r * 4 + mc, :],
                            in_=vg[r * 512 + mc * P:r * 512 + (mc + 1) * P, :])

                # ---- Q nb2/nb3 (covers the V gather) ----------------------
                if ablate != "attn":
                    with ExitStack() as p4:
                        ppQ = p4.enter_context(
                            tc.tile_pool(name="ppQb", bufs=2, space="PSUM"))
                        emit_q(2, ppQ)
                        emit_q(3, ppQ)

            # ---- attention --------------------------------------------
            with ExitStack() as btx:
                ppS = btx.enter_context(
                    tc.tile_pool(name="ppS", bufs=2, space="PSUM"))
                ppSum = btx.enter_context(
                    tc.tile_pool(name="ppSum", bufs=2, space="PSUM"))
                ppO = btx.enter_context(
                    tc.tile_pool(name="ppO", bufs=2, space="PSUM"))
                apool = btx.enter_context(tc.tile_pool(name="attnT", bufs=2))
                vpool = btx.enter_context(tc.tile_pool(name="avT", bufs=1))
                rpool = btx.enter_context(tc.tile_pool(name="rblk", bufs=2))
                opool = btx.enter_context(tc.tile_pool(name="outp", bufs=3))

                for nb in range(0 if ablate == "attn" else NL // 512):
                    q0 = nb * 512
                    aw = apool.tile([P, DC, 512], BF16)
                    r_blk = rpool.tile([P, 4], F32)
                    ps_r = ppSum.tile([P, 4], F32, name="ps_r", tag="sum")
                    for mg in range(4):
                        pss = ppS.tile([P, 1024], F32, name="pss2", tag="s")
                        for dq_c in range(DC):
                            for i in range(2):
                                m_c = mg * 2 + i
                                nc.tensor.matmul(
                                    pss[:, i * 512:(i + 1) * 512],
                                    kT[:, dq_c, m_c * P:(m_c + 1) * P],
                                    qT_all[:, dq_c, q0:q0 + 512],
                                    start=(dq_c == 0), stop=(dq_c == DC - 1))
                        nc.scalar.activation(
                            aw[:, mg * 2:(mg + 1) * 2, :],
                            pss.rearrange("p (a b) -> p a b", a=2),
                            Exp, scale=SCALE)
                    for nt in range(4):
                        for m_c in range(DC):
                            nc.tensor.matmul(
                                ps_r[:, nt:nt + 1],
                                aw[:, m_c, nt * P:(nt + 1) * P],
                                ones_bf[:, 0:1],
                                start=(m_c == 0), stop=(m_c == DC - 1))
                    nc.vector.reciprocal(r_blk, ps_r)

                    avT = vpool.tile([P, DC, 512], BF16)
                    for dv_c in range(DC):
                        psa = ppO.tile([P, 512], F32, name="psa", tag="o")
                        for m_c in range(DC):
                            nc.tensor.matmul(
                                psa, v_sb[:, m_c, dv_c * P:(dv_c + 1) * P],
                                aw[:, m_c, :],
                                start=(m_c == 0), stop=(m_c == DC - 1))
                        if dv_c % 2 == 0:
                            nc.vector.tensor_copy(out=avT[:, dv_c, :], in_=psa)
                        else:
                            nc.scalar.copy(out=avT[:, dv_c, :], in_=psa)

                    for nt in range(4):
                        for dh in range(2):
                            pso = ppO.tile([P, 512], F32, name="pso", tag="o")
                            for dv_c in range(DC):
                                nc.tensor.matmul(
                                    pso, avT[:, dv_c, nt * P:(nt + 1) * P],
                                    wo_sb[:, dv_c, dh * 512:(dh + 1) * 512],
                                    start=(dv_c == 0), stop=(dv_c == DC - 1))
                            o_t = opool.tile([P, 512], F32)
                            nc.vector.tensor_scalar(
                                out=o_t, in0=pso, scalar1=r_blk[:, nt:nt + 1],
                                scalar2=None, op0=MUL)
                            nc.vector.tensor_tensor(
                                out=o_t, in0=o_t,
                                in1=bo_sb[:, dh * 512:(dh + 1) * 512], op=ADD)
                            n0 = nb * 512 + nt * P
                            nc.sync.dma_start(
                                out=out_d[n0:n0 + P,
                                          dh * 512:(dh + 1) * 512],
                                in_=o_t)

    return nc


build_program4_layout = "split16"

_nc_cache = None


def host_prep(x, ln_g, ln_b, Wq, bq, Wk, bk, Wv, bv, Wo, bo, layout=None):
    """Fold LN affine into weights, cast to bf16, build per-core inputs."""
    bf = ml_dtypes.bfloat16
    x = np.asarray(x, np.float32)
    g = np.asarray(ln_g, np.float32)
    b_ln = np.asarray(ln_b, np.float32)
    Wq = np.asarray(Wq, np.float32); Wk = np.asarray(Wk, np.float32)
    Wv = np.asarray(Wv, np.float32); Wo = np.asarray(Wo, np.float32)

    wq_e = (g[:, None] * Wq).astype(bf)
    bq_e = (b_ln @ Wq + np.asarray(bq, np.float32)).astype(np.float32)
    g4 = np.tile(g, RATIO); b4 = np.tile(b_ln, RATIO)
    wk_e = (g4[:, None] * Wk).astype(bf)
    bk_e = (b4 @ Wk + np.asarray(bk, np.float32)).astype(np.float32)
    wv_e = (g4[:, None] * Wv).astype(bf)
    bv_e = (b4 @ Wv + np.asarray(bv, np.float32)).astype(np.float32)
    wo_e = Wo.astype(bf)
    bo_e = np.asarray(bo, np.float32)

    bq2 = np.ascontiguousarray(bq_e.reshape(DC, P).T)
    bk2 = np.ascontiguousarray(bk_e.reshape(DC, P).T)

    if layout is None:
        layout = "full" if (V2 or not SPLIT_KV) else "split"
    in_maps = []
    for c in range(N_CORES):
        bb, h = divmod(c, 2)
        if layout == "split":
            x_in = np.ascontiguousarray(x[bb, h * NL:(h + 1) * NL])
        elif layout == "split16":
            x_in = np.ascontiguousarray(
                x[bb, h * NL:(h + 1) * NL]).astype(bf)
        else:
            x_in = np.ascontiguousarray(np.roll(x[bb], -h * NL, axis=0))
            if X_BF16:
                x_in = x_in.astype(bf)
        in_maps.append({
            "x": x_in, "wq": wq_e, "wk": wk_e, "wv": wv_e, "wo": wo_e,
            "bq2": bq2, "bk2": bk2,
            "bv1": bv_e[None, :], "bo1": bo_e[None, :],
        })
    return in_maps


def gather_out(results):
    out = np.empty((4, NF, D), np.float32)
    for c in range(N_CORES):
        bb, h = divmod(c, 2)
        out[bb, h * NL:(h + 1) * NL] = results[c]["out"]
    return out


BUILD = "build_program4"
BUILD_LAYOUT = "split16"


def get_program():
    global _nc_cache
    if _nc_cache is None:
        _nc_cache = globals()[BUILD]()
        _split_multi_waits(_nc_cache)
    return _nc_cache


_runner_cache = None


def _make_runner(nc):
    """Cached-jit SPMD executor (mirrors bass2jax.run_bass_via_pjrt, but
    reusable across calls so repeat kernel() invocations don't recompile)."""
    import jax
    from jax.sharding import Mesh, PartitionSpec
    from jax.experimental.shard_map import shard_map
    import concourse.mybir as mybir
    from concourse import bass2jax
    from concourse.bass2jax import _bass_exec_p, install_neuronx_cc_hook

    install_neuronx_cc_hook()
    partition_name = (nc.partition_id_tensor.name
                      if nc.partition_id_tensor else None)
    in_names, out_names, out_avals, zero_outs = [], [], [], []
    for alloc in nc.m.functions[0].allocations:
        if not isinstance(alloc, mybir.MemoryLocationSet):
            continue
        name = alloc.memorylocations[0].name
        if alloc.kind == "ExternalInput":
            if name != partition_name:
                in_names.append(name)
        elif alloc.kind == "ExternalOutput":
            shape = tuple(alloc.tensor_shape)
            dtype = mybir.dt.np(alloc.dtype)
            out_names.append(name)
            out_avals.append(jax.core.ShapedArray(shape, dtype))
            zero_outs.append(np.zeros(shape, dtype))
    full_in_names = list(in_names) + list(out_names)
    if partition_name is not None:
        full_in_names.append(partition_name)

    def _body(*args):
        operands = list(args)
        if partition_name is not None:
            operands.append(bass2jax.partition_id_tensor())
        outs = _bass_exec_p.bind(
            *operands,
            out_avals=tuple(out_avals),
            in_names=tuple(full_in_names),
            out_names=tuple(out_names),
            lowering_input_output_aliases=(),
            sim_require_finite=True,
            sim_require_nnan=True,
            nc=nc,
        )
        return tuple(outs)

    devices = jax.devices()[:N_CORES]
    mesh = Mesh(np.asarray(devices), ("core",))
    n_in = len(in_names) + len(out_names)
    fn = jax.jit(
        shard_map(_body, mesh=mesh,
                  in_specs=(PartitionSpec("core"),) * n_in,
                  out_specs=(PartitionSpec("core"),) * len(out_names),
                  check_rep=False),
        keep_unused=True)

    def run(in_maps):
        per_core = [[np.asarray(m[name]) for name in in_names]
                    for m in in_maps]
        args = [np.concatenate([per_core[c][i] for c in range(N_CORES)],
                               axis=0) for i in range(len(in_names))]
        args += [np.zeros((N_CORES * z.shape[0], *z.shape[1:]), z.dtype)
                 for z in zero_outs]
        outs = fn(*args)
        jax.block_until_ready(outs)
        return [
            {name: np.asarray(outs[i]).reshape(N_CORES, *out_avals[i].shape)[c]
             for i, name in enumerate(out_names)}
            for c in range(N_CORES)]

    return run


def kernel(x, ln_g, ln_b, Wq, bq, Wk, bk, Wv, bv, Wo, bo):
    global _runner_cache
    nc = get_program()
    in_maps = host_prep(x, ln_g, ln_b, Wq, bq, Wk, bk, Wv, bv, Wo, bo,
                        layout=BUILD_LAYOUT)
    if _runner_cache is None:
        try:
            _runner_cache = _make_runner(nc)
        except Exception:
            from concourse.bass_utils import run_bass_kernel_spmd
            res = run_bass_kernel_spmd(nc, in_maps, list(range(N_CORES)))
            return gather_out(res.results)
    return gather_out(_runner_cache(in_maps))

